# revision 1
# baseline (speedup 1.0000x reference)
"""Trainium2 Bass kernel for AcousticTextEncoderLayer.

Reference computation (B=16, T=4096, H=512, K=9):
  w = weight_norm(weight_v, weight_g)            # per-out-channel scale
  x_masked = hidden_states * (t < len)           # zero beyond each length
  conv = conv1d(x_masked, w, same pad) + bias    # per-sample temporal conv
  y = where(t < len, conv, hidden_states)        # passthrough beyond length
  y = layernorm(y, gamma, beta); leaky_relu(y, 0.1)

Strategy: the per-(b,t) work splits into "valid" positions (t < len: conv +
LN) and "invalid" positions (t >= len: LN only).  The host packs all valid
positions of all samples into one zero-separated timeline (8 zero columns
between samples so the 9-tap conv never mixes samples), splits it evenly
across the 8 cores, and packs invalid rows into equal per-core blocks.
Every core runs the same program on its slice; the host scatters results
back.  Conv runs as 36 accumulating fp16 matmuls per 128-position tile
(9 taps x 4 input-channel chunks, fp32 PSUM accumulate) with the
weight-norm scale folded into the shipped weights; LN stats via
bn_stats/bn_aggr; normalize + leaky relu fused into one scalar-engine
activation with per-partition scale/bias (Prelu, alpha=0.1).
"""

import math

import numpy as np

B, T, H, K = 16, 4096, 512, 9
SLOPE = 0.1
EPS = 1e-5
NCORES = 8
SEG = 512          # valid-timeline columns per full segment (4 PSUM tiles)
HALO = K // 2      # 4
SEP = HALO         # zero columns between samples (taps reach <= HALO out)
CHUNKS = H // 128  # 4 input-channel chunks

XDT_NP = np.float16   # matmul operand dtype (fp16: same PE rate as bf16, more mantissa)
WARMUP_MMS = 32    # throwaway matmuls that warm the PE clock during load


def _split_sync_waits(nc, mybir, bass_rust, max_w=1):
    """walrus in this env rejects instructions carrying more than one sync
    wait.  Prefer hoisting extra waits onto the immediately preceding
    same-engine instruction when it has spare wait slots and no sem
    updates (waiting earlier on the same engine is strictly conservative,
    and update-free hosts cannot create wait/update cycles) — this avoids
    the 71ns/inst sequencer cost of a NoOp next to every matmul in the
    weight-arrival window.  Fall back to inserted NoOps otherwise."""
    def n_waits(i):
        return len(i.sync_info.on_wait or []) if i.sync_info is not None else 0

    def can_host(i):
        if i.sync_info is not None and (i.sync_info.on_update or []):
            return False
        return isinstance(i, (mybir.InstLdweights, mybir.InstNoOp))

    for fn in nc.m.functions:
        for bb in fn.blocks:
            out = []
            changed = False
            for inst in bb.instructions:
                si = inst.sync_info
                waits = list(si.on_wait or []) if si is not None else []
                if len(waits) > max_w:
                    extra, keep = waits[:-max_w], waits[-max_w:]
                    # hoist onto the preceding instruction if possible
                    if (extra and out and out[-1].engine == inst.engine
                            and can_host(out[-1])
                            and n_waits(out[-1]) < max_w):
                        prev = out[-1]
                        room = max_w - n_waits(prev)
                        moved, extra = extra[:room], extra[room:]
                        pw = (list(prev.sync_info.on_wait or [])
                              if prev.sync_info is not None else [])
                        prev.sync_info = bass_rust.SyncInfo(
                            on_wait=pw + moved, on_update=[])
                    while extra:
                        chunk, extra = extra[:max_w], extra[max_w:]
                        nop = mybir.InstNoOp(
                            name=nc.get_next_instruction_name(), ins=[], outs=[]
                        )
                        nop.engine = inst.engine
                        nop.sync_info = bass_rust.SyncInfo(
                            on_wait=chunk, on_update=[]
                        )
                        out.append(nop)
                    inst.sync_info = bass_rust.SyncInfo(
                        on_wait=keep, on_update=list(si.on_update or [])
                    )
                    changed = True
                out.append(inst)
            if changed:
                bb.instructions[:] = out


def _build_program(n_sub, nti, apply_gb, repeat=1):
    import concourse.bass as bass
    import concourse.tile as tile
    import concourse.mybir as mybir
    import bass_rust
    from contextlib import ExitStack

    f32 = mybir.dt.float32
    xdt = mybir.dt.float16

    nc = bass.Bass("TRN2", target_bir_lowering=False, debug=False,
                   num_devices=NCORES)
    Wc = n_sub * 128
    # segment widths: full SEG-wide segments plus one remainder
    seg_widths = [SEG] * (Wc // SEG)
    if Wc % SEG:
        seg_widths.append(Wc % SEG)
    nseg = len(seg_widths)
    xt = nc.dram_tensor("xt", [CHUNKS, 128, Wc + 2 * HALO], xdt,
                        kind="ExternalInput")
    wt = nc.dram_tensor("wt", [K, CHUNKS, 128, H], xdt, kind="ExternalInput")
    cb = nc.dram_tensor("cb", [1, H], f32, kind="ExternalInput")
    yv = nc.dram_tensor("yv", [Wc, H], f32, kind="ExternalOutput")
    xi = yi = None
    if nti:
        xi = nc.dram_tensor("xi", [nti, 128, H], f32, kind="ExternalInput")
        yi = nc.dram_tensor("yi", [nti, 128, H], f32, kind="ExternalOutput")
    gm = bt = None
    if apply_gb:
        gm = nc.dram_tensor("gm", [1, H], f32, kind="ExternalInput")
        bt = nc.dram_tensor("bt", [1, H], f32, kind="ExternalInput")

    AF = mybir.ActivationFunctionType
    OP = mybir.AluOpType

    with tile.TileContext(nc) as tc, ExitStack() as ctx:
        consts = ctx.enter_context(tc.tile_pool(name="consts", bufs=1))
        xpool = ctx.enter_context(tc.tile_pool(name="xpool", bufs=4))
        ipool = ctx.enter_context(tc.tile_pool(name="ipool", bufs=4))
        psum = ctx.enter_context(tc.tile_pool(name="psum", bufs=8, space="PSUM"))
        ypool = ctx.enter_context(tc.tile_pool(name="ypool", bufs=6))
        opool = ctx.enter_context(tc.tile_pool(name="opool", bufs=6))
        spool = ctx.enter_context(tc.tile_pool(name="spool", bufs=8))

        # First segment's strips go first so PE can start as soon as the
        # first weight tile lands; weights stream in in consumption order.
        strips0 = []
        for c in range(CHUNKS):
            strip = xpool.tile([128, seg_widths[0] + 2 * HALO], xdt,
                               tag=f"strip{c}")
            nc.sync.dma_start(out=strip,
                              in_=xt[c, :, 0: seg_widths[0] + 2 * HALO])
            strips0.append(strip)
        wtiles = [None] * (K * CHUNKS)
        for c in range(CHUNKS):
            for k in range(K):
                wti = consts.tile([128, H], xdt, tag=f"w{k}_{c}")
                nc.sync.dma_start(out=wti, in_=wt[k, c, :, :])
                wtiles[k * CHUNKS + c] = wti
        bias_b = consts.tile([128, H], f32, tag="bias_b")
        nc.sync.dma_start(out=bias_b, in_=cb.ap().to_broadcast((128, H)))
        gm_b = bt_b = None
        if apply_gb:
            gm_b = consts.tile([128, H], f32, tag="gm_b")
            nc.sync.dma_start(out=gm_b, in_=gm.ap().to_broadcast((128, H)))
            bt_b = consts.tile([128, H], f32, tag="bt_b")
            nc.sync.dma_start(out=bt_b, in_=bt.ap().to_broadcast((128, H)))
        eps_t = consts.tile([128, 1], f32, tag="eps")
        nc.vector.memset(eps_t, EPS)

        # Warm up the PE clock (HAM gate: 1.2 -> 2.4 GHz after ~3.4us of
        # sustained activity) with throwaway matmuls on a zeroed tile while
        # the first strips/weights are still in flight.  Results go to a
        # scratch PSUM bank nobody reads.
        if WARMUP_MMS:
            wu_src = consts.tile([128, 128], xdt, tag="wu_src")
            nc.vector.memset(wu_src, 0.0)
            wu_ps = psum.tile([128, H], f32, tag="ps")
            for _ in range(WARMUP_MMS):
                nc.tensor.matmul(wu_ps[:, 0:128], wu_src, wu_src,
                                 start=True, stop=True)

        def ln_lrelu(src, dst):
            # LayerNorm over the free dim + leaky relu, into dst.
            st = spool.tile([128, 6], f32, tag="st")
            nc.vector.bn_stats(out=st, in_=src)
            mv = spool.tile([128, 2], f32, tag="mv")
            nc.vector.bn_aggr(out=mv, in_=st)
            sd = spool.tile([128, 1], f32, tag="sd")
            nc.scalar.activation(out=sd, in_=mv[:, 1:2], func=AF.Sqrt,
                                 bias=eps_t, scale=1.0)
            rstd = spool.tile([128, 1], f32, tag="rstd")
            nc.vector.reciprocal(out=rstd, in_=sd)
            nms = spool.tile([128, 1], f32, tag="nms")
            nc.vector.tensor_scalar(out=nms, in0=mv[:, 0:1], scalar1=rstd,
                                    scalar2=-1.0, op0=OP.mult, op1=OP.mult)
            if not apply_gb:
                nc.scalar.activation(out=dst, in_=src, func=AF.Prelu,
                                     bias=nms, scale=rstd, alpha=SLOPE)
            else:
                tmp = spool.tile([128, H], f32, tag="gbtmp")
                nc.scalar.activation(out=tmp, in_=src, func=AF.Identity,
                                     bias=nms, scale=rstd)
                nc.vector.tensor_mul(out=tmp, in0=tmp, in1=gm_b)
                nc.vector.tensor_add(out=tmp, in0=tmp, in1=bt_b)
                nc.scalar.activation(out=dst, in_=tmp, func=AF.Prelu,
                                     alpha=SLOPE)

        def invalid_tile(it):
            xti = ipool.tile([128, H], f32, tag="xi")
            nc.sync.dma_start(out=xti, in_=xi[it, :, :])
            oi = opool.tile([128, H], f32, tag="oi")
            ln_lrelu(xti, oi)
            nc.sync.dma_start(out=yi[it, :, :], in_=oi)

        # Interleave invalid (LN-only) tiles among valid segments so the
        # vector/scalar engines fill PE-wait gaps.  (repeat>1 re-runs the
        # whole body with identical I/O — used only for differential
        # wall-clock timing, never for the graded kernel.)
        for _rep in range(repeat):
          done_inv = 0
          seg_start = 0
          for s, sw in enumerate(seg_widths):
            if s == 0 and _rep == 0:
                strips = strips0
            else:
                strips = []
                for c in range(CHUNKS):
                    strip = xpool.tile([128, sw + 2 * HALO], xdt,
                                       tag=f"strip{c}")
                    nc.sync.dma_start(
                        out=strip,
                        in_=xt[c, :, seg_start: seg_start + sw + 2 * HALO])
                    strips.append(strip)
            for sub in range(sw // 128):
                ps = psum.tile([128, H], f32, tag="ps")
                first = True
                for c in range(CHUNKS):
                    for k in range(K):
                        nc.tensor.matmul(
                            ps,
                            strips[c][:, sub * 128 + k: sub * 128 + k + 128],
                            wtiles[k * CHUNKS + c],
                            start=first,
                            stop=(c == CHUNKS - 1 and k == K - 1),
                        )
                        first = False
                y = ypool.tile([128, H], f32, tag="y")
                nc.vector.tensor_add(out=y, in0=ps, in1=bias_b)
                o = opool.tile([128, H], f32, tag="o")
                ln_lrelu(y, o)
                row0 = seg_start + sub * 128
                nc.sync.dma_start(out=yv[row0: row0 + 128, :], in_=o)
            seg_start += sw
            inv_target = (0 if s < 2 and s < nseg - 2 else
                          min(nti, s * nti // max(1, nseg - 3)))
            while done_inv < inv_target:
                invalid_tile(done_inv)
                done_inv += 1
        while done_inv < nti:
            invalid_tile(done_inv)
            done_inv += 1

    _split_sync_waits(nc, mybir, bass_rust)
    return nc


def _pack(hidden_states, input_lengths):
    """Build per-core packed inputs + scatter indices."""
    x = np.ascontiguousarray(np.asarray(hidden_states, dtype=np.float32))
    lens = np.asarray(input_lengths).astype(np.int64).clip(0, T)

    V = int(lens.sum())
    starts = np.zeros(B, np.int64)
    col = 0
    for b in range(B):
        starts[b] = col
        col += int(lens[b]) + SEP
    Wt = col
    n_sub = max(1, math.ceil(math.ceil(Wt / NCORES) / 128))
    Wc = n_sub * 128
    Wtot = NCORES * Wc

    XTL = np.zeros((H, HALO + Wtot + HALO), XDT_NP)
    dest = np.full(Wtot, -1, np.int64)
    for b in range(B):
        L = int(lens[b])
        s0 = int(starts[b])
        XTL[:, HALO + s0: HALO + s0 + L] = x[b, :L, :].T
        dest[s0: s0 + L] = b * T + np.arange(L, dtype=np.int64)

    xts = []
    for m in range(NCORES):
        sl = np.ascontiguousarray(XTL[:, m * Wc: m * Wc + Wc + 2 * HALO])
        xts.append(sl.reshape(CHUNKS, 128, Wc + 2 * HALO))

    # invalid rows
    inv_mask = (np.arange(T)[None, :] >= lens[:, None]).ravel()
    inv_idx = np.nonzero(inv_mask)[0]
    I = len(inv_idx)
    nti = math.ceil(I / (NCORES * 128)) if I else 0
    NI = nti * 128
    xis = None
    inv_pad = None
    if nti:
        x_flat = x.reshape(B * T, H)
        xi_all = np.zeros((NCORES * NI, H), np.float32)
        xi_all[:I] = x_flat[inv_idx]
        inv_pad = np.full(NCORES * NI, -1, np.int64)
        inv_pad[:I] = inv_idx
        xis = [np.ascontiguousarray(
            xi_all[m * NI: (m + 1) * NI].reshape(nti, 128, H))
            for m in range(NCORES)]

    return x, n_sub, Wc, dest, xts, nti, NI, inv_pad, xis


_PROGRAM_CACHE = {}


def _run(inputs, trace=False):
    from concourse.bass_utils import run_bass_kernel_spmd

    x, n_sub, Wc, dest, xts, nti, NI, inv_pad, xis = _pack(
        inputs["hidden_states"], inputs["input_lengths"])

    v = np.asarray(inputs["weight_v"], dtype=np.float32)
    g = np.asarray(inputs["weight_g"], dtype=np.float32)
    norm = np.sqrt((v * v).sum(axis=(1, 2), keepdims=True))
    w_eff = g * v / norm                                  # [H_out, H_in, K]
    wt = np.ascontiguousarray(
        w_eff.transpose(2, 1, 0)).reshape(K, CHUNKS, 128, H).astype(XDT_NP)
    cb = np.asarray(inputs["conv_bias"], np.float32).reshape(1, H)
    gamma = np.asarray(inputs["gamma"], np.float32).reshape(H)
    beta = np.asarray(inputs["beta"], np.float32).reshape(H)
    apply_gb = not (np.allclose(gamma, 1.0) and np.allclose(beta, 0.0))

    cache_key = (n_sub, nti, apply_gb)
    nc = _PROGRAM_CACHE.get(cache_key)
    if nc is None:
        nc = _build_program(n_sub, nti, apply_gb)
        _PROGRAM_CACHE[cache_key] = nc

    in_maps = []
    for m in range(NCORES):
        im = {"xt": xts[m], "wt": wt, "cb": cb}
        if nti:
            im["xi"] = xis[m]
        if apply_gb:
            im["gm"] = gamma.reshape(1, H)
            im["bt"] = beta.reshape(1, H)
        in_maps.append(im)

    res = run_bass_kernel_spmd(nc, in_maps, core_ids=list(range(NCORES)),
                               trace=trace)

    y_flat = np.empty((B * T, H), np.float32)
    for m in range(NCORES):
        yvm = np.asarray(res.results[m]["yv"])
        dm = dest[m * Wc: (m + 1) * Wc]
        sel = dm >= 0
        y_flat[dm[sel]] = yvm[sel]
        if nti:
            yim = np.asarray(res.results[m]["yi"]).reshape(NI, H)
            im_idx = inv_pad[m * NI: (m + 1) * NI]
            sel = im_idx >= 0
            y_flat[im_idx[sel]] = yim[sel]

    return y_flat.reshape(B, T, H), res


def kernel(**inputs):
    out, _ = _run(inputs, trace=False)
    return out



# revision 3
# speedup vs baseline: 1.4468x; 1.4468x over previous
"""Trainium2 Bass kernel for AcousticTextEncoderLayer.

Reference computation (B=16, T=4096, H=512, K=9):
  w = weight_norm(weight_v, weight_g)            # per-out-channel scale
  x_masked = hidden_states * (t < len)           # zero beyond each length
  conv = conv1d(x_masked, w, same pad) + bias    # per-sample temporal conv
  y = where(t < len, conv, hidden_states)        # passthrough beyond length
  y = layernorm(y, gamma, beta); leaky_relu(y, 0.1)

Strategy: the per-(b,t) work splits into "valid" positions (t < len: conv +
LN) and "invalid" positions (t >= len: LN only).  The host packs all valid
positions of all samples into one zero-separated timeline (8 zero columns
between samples so the 9-tap conv never mixes samples), splits it evenly
across the 8 cores, and packs invalid rows into equal per-core blocks.
Every core runs the same program on its slice; the host scatters results
back.

Conv runs in fp8e4m3 with DoubleRow matmuls (two 128-deep contraction
tiles per instruction at 0.5 cycles/output-column — 2x the fp16 rate).
Precision is recovered by accumulating, in the same fp32 PSUM group and
at one common scale:
  x_hi.w8  (main, 18 DoubleRow matmuls per 128-position tile)
  x_lo.w8  (exact two-term fp8 split of x: kills the x-side quant error)
  x_hi.s8  (w-residual on MTAPS of the 9 taps: kills most w-side error)
LayerNorm is scale-invariant, so the fp8 scaling (x*16, w*256) and the
matching bias*4096 need no explicit rescale.  Invalid (LN-only) rows ship
as fp16 both ways; valid outputs also return as fp16 (the 2e-2 tolerance
dwarfs fp16 rounding).  LN stats via bn_stats/bn_aggr; normalize + leaky
relu fused into one scalar-engine activation with per-partition
scale/bias (Prelu, alpha=0.1).
"""

import math

import numpy as np
import ml_dtypes

B, T, H, K = 16, 4096, 512, 9
SLOPE = 0.1
EPS = 1e-5
NCORES = 8
SEG = 512          # valid-timeline columns per full segment (4 PSUM tiles)
HALO = K // 2      # 4
SEP = HALO         # zero columns between samples (taps reach <= HALO out)
NPAIR = 2          # DoubleRow chunk pairs (2 x 256 input channels)

SX = 16.0          # fp8 scale for x (LayerNorm absorbs it)
SW = 256.0         # fp8 scale for w
MTAPS = 6          # taps with w-residual correction (of K)
E4 = ml_dtypes.float8_e4m3
WARMUP_MMS = 36    # throwaway matmuls that warm the PE clock during load


def _split_sync_waits(nc, mybir, bass_rust, max_w=1):
    """walrus in this env rejects instructions carrying more than one sync
    wait.  Prefer hoisting extra waits onto the immediately preceding
    same-engine instruction when it has spare wait slots and no sem
    updates (waiting earlier on the same engine is strictly conservative,
    and update-free hosts cannot create wait/update cycles) — this avoids
    the 71ns/inst sequencer cost of a NoOp next to every matmul in the
    weight-arrival window.  Fall back to inserted NoOps otherwise."""
    def n_waits(i):
        return len(i.sync_info.on_wait or []) if i.sync_info is not None else 0

    def can_host(i):
        if i.sync_info is not None and (i.sync_info.on_update or []):
            return False
        return isinstance(i, (mybir.InstLdweights, mybir.InstNoOp))

    for fn in nc.m.functions:
        for bb in fn.blocks:
            out = []
            changed = False
            for inst in bb.instructions:
                si = inst.sync_info
                waits = list(si.on_wait or []) if si is not None else []
                if len(waits) > max_w:
                    extra, keep = waits[:-max_w], waits[-max_w:]
                    # hoist onto the preceding instruction if possible
                    if (extra and out and out[-1].engine == inst.engine
                            and can_host(out[-1])
                            and n_waits(out[-1]) < max_w):
                        prev = out[-1]
                        room = max_w - n_waits(prev)
                        moved, extra = extra[:room], extra[room:]
                        pw = (list(prev.sync_info.on_wait or [])
                              if prev.sync_info is not None else [])
                        prev.sync_info = bass_rust.SyncInfo(
                            on_wait=pw + moved, on_update=[])
                    while extra:
                        chunk, extra = extra[:max_w], extra[max_w:]
                        nop = mybir.InstNoOp(
                            name=nc.get_next_instruction_name(), ins=[], outs=[]
                        )
                        nop.engine = inst.engine
                        nop.sync_info = bass_rust.SyncInfo(
                            on_wait=chunk, on_update=[]
                        )
                        out.append(nop)
                    inst.sync_info = bass_rust.SyncInfo(
                        on_wait=keep, on_update=list(si.on_update or [])
                    )
                    changed = True
                out.append(inst)
            if changed:
                bb.instructions[:] = out


def _build_program(n_sub, nti, apply_gb, repeat=1):
    import concourse.bass as bass
    import concourse.tile as tile
    import concourse.mybir as mybir
    import bass_rust
    from contextlib import ExitStack

    f32 = mybir.dt.float32
    f16 = mybir.dt.float16
    f8 = mybir.dt.float8e4
    DR = mybir.MatmulPerfMode.DoubleRow

    nc = bass.Bass("TRN2", target_bir_lowering=False, debug=False,
                   num_devices=NCORES)
    Wc = n_sub * 128
    # segment widths: full SEG-wide segments plus one remainder
    seg_widths = [SEG] * (Wc // SEG)
    if Wc % SEG:
        seg_widths.append(Wc % SEG)
    nseg = len(seg_widths)
    # x planes: [plane(hi/lo), pair, partition, slot, column]
    xx = nc.dram_tensor("xx", [2, NPAIR, 128, 2, Wc + 2 * HALO], f8,
                        kind="ExternalInput")
    wt = nc.dram_tensor("wt", [K, NPAIR, 128, 2, H], f8, kind="ExternalInput")
    st = None
    if MTAPS:
        st = nc.dram_tensor("st", [MTAPS, NPAIR, 128, 2, H], f8,
                            kind="ExternalInput")
    cb = nc.dram_tensor("cb", [1, H], f32, kind="ExternalInput")
    yv = nc.dram_tensor("yv", [Wc, H], f16, kind="ExternalOutput")
    xi = yi = None
    if nti:
        xi = nc.dram_tensor("xi", [nti, 128, H], f16, kind="ExternalInput")
        yi = nc.dram_tensor("yi", [nti, 128, H], f16, kind="ExternalOutput")
    gm = bt = None
    if apply_gb:
        gm = nc.dram_tensor("gm", [1, H], f32, kind="ExternalInput")
        bt = nc.dram_tensor("bt", [1, H], f32, kind="ExternalInput")

    AF = mybir.ActivationFunctionType
    OP = mybir.AluOpType

    with tile.TileContext(nc) as tc, ExitStack() as ctx:
        consts = ctx.enter_context(tc.tile_pool(name="consts", bufs=1))
        xpool = ctx.enter_context(tc.tile_pool(name="xpool", bufs=8))
        ipool = ctx.enter_context(tc.tile_pool(name="ipool", bufs=4))
        psum = ctx.enter_context(tc.tile_pool(name="psum", bufs=8, space="PSUM"))
        ypool = ctx.enter_context(tc.tile_pool(name="ypool", bufs=6))
        opool = ctx.enter_context(tc.tile_pool(name="opool", bufs=6))
        spool = ctx.enter_context(tc.tile_pool(name="spool", bufs=8))

        def seg_strips(seg_start, sw):
            # 4 strips per segment: (plane hi/lo) x (chunk pair), each
            # [128, 2(slot), sw + 2*HALO] fp8.  The slot-dim stride of a
            # DoubleRow ldweights AP must be a multiple of 16 bytes, so the
            # tile width is padded up (the pad columns are never read).
            w_used = sw + 2 * HALO
            w_pad = (w_used + 15) // 16 * 16
            strips = []
            for plane in range(2):
                row = []
                for pr in range(NPAIR):
                    strip = xpool.tile([128, 2, w_pad], f8,
                                       tag=f"strip{plane}_{pr}")
                    nc.sync.dma_start(
                        out=strip[:, :, 0:w_used],
                        in_=xx[plane, pr, :, :,
                               seg_start: seg_start + w_used])
                    row.append(strip)
                strips.append(row)
            return strips

        # First segment's strips go first so PE can start as soon as the
        # first weight tile lands; weights stream in in consumption order.
        strips0 = seg_strips(0, seg_widths[0])
        wtiles = [[None] * K for _ in range(NPAIR)]
        for pr in range(NPAIR):
            for k in range(K):
                wti = consts.tile([128, 2, H], f8, tag=f"w{k}_{pr}")
                nc.sync.dma_start(out=wti, in_=wt[k, pr, :, :, :])
                wtiles[pr][k] = wti
        stiles = [[None] * MTAPS for _ in range(NPAIR)]
        for pr in range(NPAIR):
            for k in range(MTAPS):
                sti = consts.tile([128, 2, H], f8, tag=f"s{k}_{pr}")
                nc.sync.dma_start(out=sti, in_=st[k, pr, :, :, :])
                stiles[pr][k] = sti
        bias_b = consts.tile([128, H], f32, tag="bias_b")
        nc.sync.dma_start(out=bias_b, in_=cb.ap().to_broadcast((128, H)))
        gm_b = bt_b = None
        if apply_gb:
            gm_b = consts.tile([128, H], f32, tag="gm_b")
            nc.sync.dma_start(out=gm_b, in_=gm.ap().to_broadcast((128, H)))
            bt_b = consts.tile([128, H], f32, tag="bt_b")
            nc.sync.dma_start(out=bt_b, in_=bt.ap().to_broadcast((128, H)))
        eps_t = consts.tile([128, 1], f32, tag="eps")
        nc.vector.memset(eps_t, EPS)

        # Warm up the PE clock (HAM gate: 1.2 -> 2.4 GHz after ~3us of
        # sustained activity) with throwaway matmuls on a zeroed tile while
        # the first strips/weights are still in flight.  Results go to a
        # scratch PSUM bank nobody reads.
        if WARMUP_MMS:
            wu_src = consts.tile([128, 2, 128], f8, tag="wu_src")
            nc.vector.memset(wu_src, 0.0)
            wu_mov = consts.tile([128, 2, H], f8, tag="wu_mov")
            nc.vector.memset(wu_mov, 0.0)
            wu_ps = psum.tile([128, H], f32, tag="ps")
            for _ in range(WARMUP_MMS):
                nc.tensor.matmul(wu_ps, wu_src, wu_mov,
                                 start=True, stop=True, perf_mode=DR)

        def ln_lrelu(src, dst):
            # LayerNorm over the free dim + leaky relu, into dst.
            stt = spool.tile([128, 6], f32, tag="st")
            nc.vector.bn_stats(out=stt, in_=src)
            mv = spool.tile([128, 2], f32, tag="mv")
            nc.vector.bn_aggr(out=mv, in_=stt)
            sd = spool.tile([128, 1], f32, tag="sd")
            nc.scalar.activation(out=sd, in_=mv[:, 1:2], func=AF.Sqrt,
                                 bias=eps_t, scale=1.0)
            rstd = spool.tile([128, 1], f32, tag="rstd")
            nc.vector.reciprocal(out=rstd, in_=sd)
            nms = spool.tile([128, 1], f32, tag="nms")
            nc.vector.tensor_scalar(out=nms, in0=mv[:, 0:1], scalar1=rstd,
                                    scalar2=-1.0, op0=OP.mult, op1=OP.mult)
            if not apply_gb:
                nc.scalar.activation(out=dst, in_=src, func=AF.Prelu,
                                     bias=nms, scale=rstd, alpha=SLOPE)
            else:
                tmp = spool.tile([128, H], f32, tag="gbtmp")
                nc.scalar.activation(out=tmp, in_=src, func=AF.Identity,
                                     bias=nms, scale=rstd)
                nc.vector.tensor_mul(out=tmp, in0=tmp, in1=gm_b)
                nc.vector.tensor_add(out=tmp, in0=tmp, in1=bt_b)
                nc.scalar.activation(out=dst, in_=tmp, func=AF.Prelu,
                                     alpha=SLOPE)

        def invalid_tile(it):
            xti = ipool.tile([128, H], f16, tag="xi")
            nc.sync.dma_start(out=xti, in_=xi[it, :, :])
            oi = opool.tile([128, H], f16, tag="oi")
            ln_lrelu(xti, oi)
            nc.sync.dma_start(out=yi[it, :, :], in_=oi)

        # Interleave invalid (LN-only) tiles among valid segments so the
        # vector/scalar engines fill PE-wait gaps.  (repeat>1 re-runs the
        # whole body with identical I/O — used only for differential
        # wall-clock timing, never for the graded kernel.)
        for _rep in range(repeat):
          done_inv = 0
          seg_start = 0
          for s, sw in enumerate(seg_widths):
            if s == 0 and _rep == 0:
                strips = strips0
            else:
                strips = seg_strips(seg_start, sw)
            for sub in range(sw // 128):
                ps = psum.tile([128, H], f32, tag="ps")
                # one fp32 PSUM accumulation group per tile:
                #   x_hi.w8 (all taps) + x_lo.w8 (all taps) + x_hi.s8
                seq = []
                for pr in range(NPAIR):
                    for k in range(K):
                        seq.append((strips[0][pr], k, wtiles[pr][k]))
                for pr in range(NPAIR):
                    for k in range(K):
                        seq.append((strips[1][pr], k, wtiles[pr][k]))
                for pr in range(NPAIR):
                    for k in range(MTAPS):
                        seq.append((strips[0][pr], k, stiles[pr][k]))
                last = len(seq) - 1
                for i, (strip, k, wti) in enumerate(seq):
                    nc.tensor.matmul(
                        ps,
                        strip[:, :, sub * 128 + k: sub * 128 + k + 128],
                        wti,
                        start=(i == 0),
                        stop=(i == last),
                        perf_mode=DR,
                    )
                y = ypool.tile([128, H], f32, tag="y")
                nc.vector.tensor_add(out=y, in0=ps, in1=bias_b)
                o = opool.tile([128, H], f16, tag="o")
                ln_lrelu(y, o)
                row0 = seg_start + sub * 128
                nc.sync.dma_start(out=yv[row0: row0 + 128, :], in_=o)
            seg_start += sw
            inv_target = (0 if s < 2 and s < nseg - 2 else
                          min(nti, s * nti // max(1, nseg - 3)))
            while done_inv < inv_target:
                invalid_tile(done_inv)
                done_inv += 1
        while done_inv < nti:
            invalid_tile(done_inv)
            done_inv += 1

    _split_sync_waits(nc, mybir, bass_rust)
    return nc


def _to_pairs(a):
    """[512, ...cols] -> [pair, 128, slot, ...cols] with
    channel = pair*256 + slot*128 + partition."""
    s = a.shape[1:]
    return np.ascontiguousarray(
        a.reshape(NPAIR, 2, 128, *s).transpose(0, 2, 1, 3))


def _pack(hidden_states, input_lengths):
    """Build per-core packed fp8 inputs + scatter indices."""
    x = np.ascontiguousarray(np.asarray(hidden_states, dtype=np.float32))
    lens = np.asarray(input_lengths).astype(np.int64).clip(0, T)

    starts = np.zeros(B, np.int64)
    col = 0
    for b in range(B):
        starts[b] = col
        col += int(lens[b]) + SEP
    Wt = col
    n_sub = max(1, math.ceil(math.ceil(Wt / NCORES) / 128))
    Wc = n_sub * 128
    Wtot = NCORES * Wc

    XTL = np.zeros((H, HALO + Wtot + HALO), np.float32)
    dest = np.full(Wtot, -1, np.int64)
    for b in range(B):
        L = int(lens[b])
        s0 = int(starts[b])
        XTL[:, HALO + s0: HALO + s0 + L] = x[b, :L, :].T
        dest[s0: s0 + L] = b * T + np.arange(L, dtype=np.int64)

    XTL *= SX
    hi8 = XTL.astype(E4)
    lo8 = (XTL - hi8.astype(np.float32)).astype(E4)

    xts = []
    for m in range(NCORES):
        sl_h = _to_pairs(hi8[:, m * Wc: m * Wc + Wc + 2 * HALO])
        sl_l = _to_pairs(lo8[:, m * Wc: m * Wc + Wc + 2 * HALO])
        xts.append(np.ascontiguousarray(np.stack([sl_h, sl_l])))

    # invalid rows
    inv_mask = (np.arange(T)[None, :] >= lens[:, None]).ravel()
    inv_idx = np.nonzero(inv_mask)[0]
    I = len(inv_idx)
    nti = math.ceil(I / (NCORES * 128)) if I else 0
    NI = nti * 128
    xis = None
    inv_pad = None
    if nti:
        x_flat = x.reshape(B * T, H)
        xi_all = np.zeros((NCORES * NI, H), np.float16)
        xi_all[:I] = x_flat[inv_idx]
        inv_pad = np.full(NCORES * NI, -1, np.int64)
        inv_pad[:I] = inv_idx
        xis = [np.ascontiguousarray(
            xi_all[m * NI: (m + 1) * NI].reshape(nti, 128, H))
            for m in range(NCORES)]

    return x, n_sub, Wc, dest, xts, nti, NI, inv_pad, xis


def _prep_weights(inputs):
    """Quantize weight-norm'ed conv weights to fp8 main + residual."""
    v = np.asarray(inputs["weight_v"], dtype=np.float32)
    g = np.asarray(inputs["weight_g"], dtype=np.float32)
    norm = np.sqrt((v * v).sum(axis=(1, 2), keepdims=True))
    w_eff = (g * v / norm) * SW                           # [H_out, H_in, K]
    wk = np.ascontiguousarray(w_eff.transpose(2, 1, 0))   # [K, H_in, H_out]
    w8 = wk.astype(E4)
    s8 = (wk[:MTAPS] - w8[:MTAPS].astype(np.float32)).astype(E4)
    wt = np.ascontiguousarray(
        np.stack([_to_pairs(w8[k]) for k in range(K)]))
    st = np.ascontiguousarray(
        np.stack([_to_pairs(s8[k]) for k in range(MTAPS)])) if MTAPS else None
    cb = (np.asarray(inputs["conv_bias"], np.float32) * SX * SW).reshape(1, H)
    return wt, st, cb


_PROGRAM_CACHE = {}


def _prep(inputs):
    """Pack inputs; return (program, per-core in_maps, scatter metadata)."""
    x, n_sub, Wc, dest, xts, nti, NI, inv_pad, xis = _pack(
        inputs["hidden_states"], inputs["input_lengths"])
    wt, st, cb = _prep_weights(inputs)
    gamma = np.asarray(inputs["gamma"], np.float32).reshape(H)
    beta = np.asarray(inputs["beta"], np.float32).reshape(H)
    apply_gb = not (np.allclose(gamma, 1.0) and np.allclose(beta, 0.0))

    cache_key = (n_sub, nti, apply_gb)
    nc = _PROGRAM_CACHE.get(cache_key)
    if nc is None:
        nc = _build_program(n_sub, nti, apply_gb)
        _PROGRAM_CACHE[cache_key] = nc

    in_maps = []
    for m in range(NCORES):
        im = {"xx": xts[m], "wt": wt, "cb": cb}
        if MTAPS:
            im["st"] = st
        if nti:
            im["xi"] = xis[m]
        if apply_gb:
            im["gm"] = gamma.reshape(1, H)
            im["bt"] = beta.reshape(1, H)
        in_maps.append(im)
    meta = (Wc, dest, nti, NI, inv_pad)
    return nc, in_maps, meta


def _run(inputs, trace=False):
    from concourse.bass_utils import run_bass_kernel_spmd

    nc, in_maps, meta = _prep(inputs)
    Wc, dest, nti, NI, inv_pad = meta

    res = run_bass_kernel_spmd(nc, in_maps, core_ids=list(range(NCORES)),
                               trace=trace)

    y_flat = np.empty((B * T, H), np.float32)
    for m in range(NCORES):
        yvm = np.asarray(res.results[m]["yv"]).astype(np.float32)
        dm = dest[m * Wc: (m + 1) * Wc]
        sel = dm >= 0
        y_flat[dm[sel]] = yvm[sel]
        if nti:
            yim = np.asarray(res.results[m]["yi"]).astype(
                np.float32).reshape(NI, H)
            im_idx = inv_pad[m * NI: (m + 1) * NI]
            sel = im_idx >= 0
            y_flat[im_idx[sel]] = yim[sel]

    return y_flat.reshape(B, T, H), res


def kernel(**inputs):
    out, _ = _run(inputs, trace=False)
    return out


# revision 8
# speedup vs baseline: 1.5943x; 1.1019x over previous
"""Trainium2 Bass kernel for AcousticTextEncoderLayer.

Reference computation (B=16, T=4096, H=512, K=9):
  w = weight_norm(weight_v, weight_g)            # per-out-channel scale
  x_masked = hidden_states * (t < len)           # zero beyond each length
  conv = conv1d(x_masked, w, same pad) + bias    # per-sample temporal conv
  y = where(t < len, conv, hidden_states)        # passthrough beyond length
  y = layernorm(y, gamma, beta); leaky_relu(y, 0.1)

Strategy: the per-(b,t) work splits into "valid" positions (t < len: conv +
LN) and "invalid" positions (t >= len: LN only).  The host packs all valid
positions of all samples into one zero-separated timeline (8 zero columns
between samples so the 9-tap conv never mixes samples), splits it evenly
across the 8 cores, and packs invalid rows into equal per-core blocks.
Every core runs the same program on its slice; the host scatters results
back.

Conv runs in fp8e4m3 with DoubleRow matmuls (two 128-deep contraction
tiles per instruction at 0.5 cycles/output-column — 2x the fp16 rate).
Precision is recovered by accumulating, in the same fp32 PSUM group and
at one common scale:
  x_hi.w8  (main, 18 DoubleRow matmuls per 128-position tile)
  x_lo.w8  (exact two-term fp8 split of x: kills the x-side quant error)
  x_hi.s8  (w-residual on MTAPS of the 9 taps: kills most w-side error)
LayerNorm is scale-invariant, so the fp8 scaling (x*16, w*256) and the
matching bias*4096 need no explicit rescale.  Invalid (LN-only) rows ship
as fp16 both ways; valid outputs also return as fp16 (the 2e-2 tolerance
dwarfs fp16 rounding).  LN stats via bn_stats/bn_aggr; normalize + leaky
relu fused into one scalar-engine activation with per-partition
scale/bias (Prelu, alpha=0.1).
"""

import math

import numpy as np
import ml_dtypes

B, T, H, K = 16, 4096, 512, 9
SLOPE = 0.1
EPS = 1e-5
NCORES = 8
SEG = 512          # valid-timeline columns per full segment (4 PSUM tiles)
HALO = K // 2      # 4
SEP = HALO         # zero columns between samples (taps reach <= HALO out)
NPAIR = 2          # DoubleRow chunk pairs (2 x 256 input channels)

SX = 8.0           # fp8 scale for x (LayerNorm absorbs it; small enough
                   # that scaled conv values stay well inside fp16 range)
SW = 256.0         # fp8 scale for w
MTAPS = 4          # taps with w-residual correction (of K)
E4 = ml_dtypes.float8_e4m3
WARMUP_MMS = 24    # throwaway matmuls that warm the PE clock during load


def _split_sync_waits(nc, mybir, bass_rust, max_w=1):
    """walrus in this env rejects instructions carrying more than one sync
    wait.  Prefer hoisting extra waits onto the immediately preceding
    same-engine instruction when it has spare wait slots and no sem
    updates (waiting earlier on the same engine is strictly conservative,
    and update-free hosts cannot create wait/update cycles) — this avoids
    the 71ns/inst sequencer cost of a NoOp next to every matmul in the
    weight-arrival window.  Fall back to inserted NoOps otherwise."""
    def n_waits(i):
        return len(i.sync_info.on_wait or []) if i.sync_info is not None else 0

    def can_host(i):
        if i.sync_info is not None and (i.sync_info.on_update or []):
            return False
        return isinstance(i, (mybir.InstLdweights, mybir.InstNoOp))

    for fn in nc.m.functions:
        for bb in fn.blocks:
            out = []
            changed = False
            for inst in bb.instructions:
                si = inst.sync_info
                waits = list(si.on_wait or []) if si is not None else []
                if len(waits) > max_w:
                    extra, keep = waits[:-max_w], waits[-max_w:]
                    # hoist onto the preceding instruction if possible
                    if (extra and out and out[-1].engine == inst.engine
                            and can_host(out[-1])
                            and n_waits(out[-1]) < max_w):
                        prev = out[-1]
                        room = max_w - n_waits(prev)
                        moved, extra = extra[:room], extra[room:]
                        pw = (list(prev.sync_info.on_wait or [])
                              if prev.sync_info is not None else [])
                        prev.sync_info = bass_rust.SyncInfo(
                            on_wait=pw + moved, on_update=[])
                    while extra:
                        chunk, extra = extra[:max_w], extra[max_w:]
                        nop = mybir.InstNoOp(
                            name=nc.get_next_instruction_name(), ins=[], outs=[]
                        )
                        nop.engine = inst.engine
                        nop.sync_info = bass_rust.SyncInfo(
                            on_wait=chunk, on_update=[]
                        )
                        out.append(nop)
                    inst.sync_info = bass_rust.SyncInfo(
                        on_wait=keep, on_update=list(si.on_update or [])
                    )
                    changed = True
                out.append(inst)
            if changed:
                bb.instructions[:] = out


def _build_program(n_sub, nti, apply_gb, repeat=1):
    import concourse.bass as bass
    import concourse.tile as tile
    import concourse.mybir as mybir
    import bass_rust
    from contextlib import ExitStack

    f32 = mybir.dt.float32
    f16 = mybir.dt.float16
    f8 = mybir.dt.float8e4
    DR = mybir.MatmulPerfMode.DoubleRow

    nc = bass.Bass("TRN2", target_bir_lowering=False, debug=False,
                   num_devices=NCORES)
    Wc = n_sub * 128
    # segment widths: full SEG-wide segments plus one remainder
    seg_widths = [SEG] * (Wc // SEG)
    if Wc % SEG:
        seg_widths.append(Wc % SEG)
    nseg = len(seg_widths)
    # x planes: [plane(hi/lo), pair, partition, slot, column]
    xx = nc.dram_tensor("xx", [2, NPAIR, 128, 2, Wc + 2 * HALO], f8,
                        kind="ExternalInput")
    wt = nc.dram_tensor("wt", [NPAIR, 128, K, 2, H], f8, kind="ExternalInput")
    st = None
    if MTAPS:
        st = nc.dram_tensor("st", [NPAIR, 128, MTAPS, 2, H], f8,
                            kind="ExternalInput")
    cb = nc.dram_tensor("cb", [1, H], f16, kind="ExternalInput")
    yv = nc.dram_tensor("yv", [Wc, H], f16, kind="ExternalOutput")
    xi = yi = None
    if nti:
        xi = nc.dram_tensor("xi", [nti, 128, H], f16, kind="ExternalInput")
        yi = nc.dram_tensor("yi", [nti, 128, H], f16, kind="ExternalOutput")
    gm = bt = None
    if apply_gb:
        gm = nc.dram_tensor("gm", [1, H], f32, kind="ExternalInput")
        bt = nc.dram_tensor("bt", [1, H], f32, kind="ExternalInput")

    AF = mybir.ActivationFunctionType
    OP = mybir.AluOpType

    with tile.TileContext(nc) as tc, ExitStack() as ctx:
        consts = ctx.enter_context(tc.tile_pool(name="consts", bufs=1))
        xpool = ctx.enter_context(tc.tile_pool(name="xpool", bufs=8))
        ipool = ctx.enter_context(tc.tile_pool(name="ipool", bufs=4))
        psum = ctx.enter_context(tc.tile_pool(name="psum", bufs=8, space="PSUM"))
        ypool = ctx.enter_context(tc.tile_pool(name="ypool", bufs=6))
        opool = ctx.enter_context(tc.tile_pool(name="opool", bufs=6))
        spool = ctx.enter_context(tc.tile_pool(name="spool", bufs=8))

        def seg_strips(seg_start, sw):
            # 4 strips per segment: (plane hi/lo) x (chunk pair), each
            # [128, 2(slot), sw + 2*HALO] fp8.  The slot-dim stride of a
            # DoubleRow ldweights AP must be a multiple of 16 bytes, so the
            # tile width is padded up (the pad columns are never read).
            w_used = sw + 2 * HALO
            w_pad = (w_used + 15) // 16 * 16
            strips = []
            for plane in range(2):
                row = []
                for pr in range(NPAIR):
                    strip = xpool.tile([128, 2, w_pad], f8,
                                       tag=f"strip{plane}_{pr}")
                    nc.sync.dma_start(
                        out=strip[:, :, 0:w_used],
                        in_=xx[plane, pr, :, :,
                               seg_start: seg_start + w_used])
                    row.append(strip)
                strips.append(row)
            return strips

        # First segment's strips go first so PE can start as soon as the
        # first weight block lands; weights come as one DMA per chunk pair
        # (streamed in consumption order) so the PE is not gated on dozens
        # of small transfers.
        strips0 = seg_strips(0, seg_widths[0])
        wblocks = []
        for pr in range(NPAIR):
            wbl = consts.tile([128, K, 2, H], f8, tag=f"wb{pr}")
            nc.sync.dma_start(out=wbl, in_=wt[pr, :, :, :, :])
            wblocks.append(wbl)
        sblocks = []
        for pr in range(NPAIR):
            if not MTAPS:
                break
            sbl = consts.tile([128, MTAPS, 2, H], f8, tag=f"sb{pr}")
            nc.sync.dma_start(out=sbl, in_=st[pr, :, :, :, :])
            sblocks.append(sbl)
        wtiles = [[wblocks[pr][:, k, :, :] for k in range(K)]
                  for pr in range(NPAIR)]
        stiles = [[sblocks[pr][:, k, :, :] for k in range(MTAPS)]
                  for pr in range(NPAIR)] if MTAPS else []
        bias_b = consts.tile([128, H], f16, tag="bias_b")
        nc.sync.dma_start(out=bias_b, in_=cb.ap().to_broadcast((128, H)))
        gm_b = bt_b = None
        if apply_gb:
            gm_b = consts.tile([128, H], f32, tag="gm_b")
            nc.sync.dma_start(out=gm_b, in_=gm.ap().to_broadcast((128, H)))
            bt_b = consts.tile([128, H], f32, tag="bt_b")
            nc.sync.dma_start(out=bt_b, in_=bt.ap().to_broadcast((128, H)))
        eps_t = consts.tile([128, 1], f32, tag="eps")
        nc.vector.memset(eps_t, EPS)

        # Warm up the PE clock (HAM gate: 1.2 -> 2.4 GHz after ~3us of
        # sustained activity) with throwaway matmuls on a zeroed tile while
        # the first strips/weights are still in flight.  Results go to a
        # scratch PSUM bank nobody reads.
        if WARMUP_MMS:
            wu_src = consts.tile([128, 2, 128], f8, tag="wu_src")
            nc.vector.memset(wu_src, 0.0)
            wu_mov = consts.tile([128, 2, H], f8, tag="wu_mov")
            nc.vector.memset(wu_mov, 0.0)
            wu_ps = psum.tile([128, H], f32, tag="ps")
            for _ in range(WARMUP_MMS):
                nc.tensor.matmul(wu_ps, wu_src, wu_mov,
                                 start=True, stop=True, perf_mode=DR)

        def ln_lrelu(src, dst):
            # LayerNorm over the free dim + leaky relu, into dst.
            stt = spool.tile([128, 6], f32, tag="st")
            nc.vector.bn_stats(out=stt, in_=src)
            mv = spool.tile([128, 2], f32, tag="mv")
            nc.vector.bn_aggr(out=mv, in_=stt)
            sd = spool.tile([128, 1], f32, tag="sd")
            nc.scalar.activation(out=sd, in_=mv[:, 1:2], func=AF.Sqrt,
                                 bias=eps_t, scale=1.0)
            rstd = spool.tile([128, 1], f32, tag="rstd")
            nc.vector.reciprocal(out=rstd, in_=sd)
            nms = spool.tile([128, 1], f32, tag="nms")
            nc.vector.tensor_scalar(out=nms, in0=mv[:, 0:1], scalar1=rstd,
                                    scalar2=-1.0, op0=OP.mult, op1=OP.mult)
            if not apply_gb:
                nc.scalar.activation(out=dst, in_=src, func=AF.Prelu,
                                     bias=nms, scale=rstd, alpha=SLOPE)
            else:
                tmp = spool.tile([128, H], f32, tag="gbtmp")
                nc.scalar.activation(out=tmp, in_=src, func=AF.Identity,
                                     bias=nms, scale=rstd)
                nc.vector.tensor_mul(out=tmp, in0=tmp, in1=gm_b)
                nc.vector.tensor_add(out=tmp, in0=tmp, in1=bt_b)
                nc.scalar.activation(out=dst, in_=tmp, func=AF.Prelu,
                                     alpha=SLOPE)

        def invalid_tile(it):
            xti = ipool.tile([128, H], f16, tag="xi")
            nc.sync.dma_start(out=xti, in_=xi[it, :, :])
            oi = opool.tile([128, H], f16, tag="oi")
            ln_lrelu(xti, oi)
            nc.sync.dma_start(out=yi[it, :, :], in_=oi)

        # Interleave invalid (LN-only) tiles among valid segments so the
        # vector/scalar engines fill PE-wait gaps.  (repeat>1 re-runs the
        # whole body with identical I/O — used only for differential
        # wall-clock timing, never for the graded kernel.)
        for _rep in range(repeat):
          done_inv = 0
          seg_start = 0
          for s, sw in enumerate(seg_widths):
            if s == 0 and _rep == 0:
                strips = strips0
            else:
                strips = seg_strips(seg_start, sw)
            for sub in range(sw // 128):
                ps = psum.tile([128, H], f32, tag="ps")
                # one fp32 PSUM accumulation group per tile:
                #   x_hi.w8 (all taps) + x_lo.w8 (all taps) + x_hi.s8
                seq = []
                for pr in range(NPAIR):
                    for k in range(K):
                        seq.append((strips[0][pr], k, wtiles[pr][k]))
                for pr in range(NPAIR):
                    for k in range(K):
                        seq.append((strips[1][pr], k, wtiles[pr][k]))
                for pr in range(NPAIR):
                    for k in range(MTAPS):
                        seq.append((strips[0][pr], k, stiles[pr][k]))
                last = len(seq) - 1
                for i, (strip, k, wti) in enumerate(seq):
                    nc.tensor.matmul(
                        ps,
                        strip[:, :, sub * 128 + k: sub * 128 + k + 128],
                        wti,
                        start=(i == 0),
                        stop=(i == last),
                        perf_mode=DR,
                    )
                y = ypool.tile([128, H], f16, tag="y")
                nc.vector.tensor_add(out=y, in0=ps, in1=bias_b)
                o = opool.tile([128, H], f16, tag="o")
                ln_lrelu(y, o)
                row0 = seg_start + sub * 128
                nc.sync.dma_start(out=yv[row0: row0 + 128, :], in_=o)
            seg_start += sw
            inv_target = (0 if s < 2 and s < nseg - 2 else
                          min(nti, s * nti // max(1, nseg - 3)))
            while done_inv < inv_target:
                invalid_tile(done_inv)
                done_inv += 1
        while done_inv < nti:
            invalid_tile(done_inv)
            done_inv += 1

    _split_sync_waits(nc, mybir, bass_rust)
    return nc


def _to_pairs(a):
    """[512, ...cols] -> [pair, 128, slot, ...cols] with
    channel = pair*256 + slot*128 + partition."""
    s = a.shape[1:]
    return np.ascontiguousarray(
        a.reshape(NPAIR, 2, 128, *s).transpose(0, 2, 1, 3))


def _pack(hidden_states, input_lengths):
    """Build per-core packed fp8 inputs + scatter indices."""
    x = np.ascontiguousarray(np.asarray(hidden_states, dtype=np.float32))
    lens = np.asarray(input_lengths).astype(np.int64).clip(0, T)

    starts = np.zeros(B, np.int64)
    col = 0
    for b in range(B):
        starts[b] = col
        col += int(lens[b]) + SEP
    Wt = col
    n_sub = max(1, math.ceil(math.ceil(Wt / NCORES) / 128))
    Wc = n_sub * 128
    Wtot = NCORES * Wc

    XTL = np.zeros((H, HALO + Wtot + HALO), np.float32)
    dest = np.full(Wtot, -1, np.int64)
    for b in range(B):
        L = int(lens[b])
        s0 = int(starts[b])
        XTL[:, HALO + s0: HALO + s0 + L] = x[b, :L, :].T
        dest[s0: s0 + L] = b * T + np.arange(L, dtype=np.int64)

    XTL *= SX
    hi8 = XTL.astype(E4)
    lo8 = (XTL - hi8.astype(np.float32)).astype(E4)

    xts = []
    for m in range(NCORES):
        sl_h = _to_pairs(hi8[:, m * Wc: m * Wc + Wc + 2 * HALO])
        sl_l = _to_pairs(lo8[:, m * Wc: m * Wc + Wc + 2 * HALO])
        xts.append(np.ascontiguousarray(np.stack([sl_h, sl_l])))

    # invalid rows
    inv_mask = (np.arange(T)[None, :] >= lens[:, None]).ravel()
    inv_idx = np.nonzero(inv_mask)[0]
    I = len(inv_idx)
    nti = math.ceil(I / (NCORES * 128)) if I else 0
    NI = nti * 128
    xis = None
    inv_pad = None
    if nti:
        x_flat = x.reshape(B * T, H)
        xi_all = np.zeros((NCORES * NI, H), np.float16)
        xi_all[:I] = x_flat[inv_idx]
        inv_pad = np.full(NCORES * NI, -1, np.int64)
        inv_pad[:I] = inv_idx
        xis = [np.ascontiguousarray(
            xi_all[m * NI: (m + 1) * NI].reshape(nti, 128, H))
            for m in range(NCORES)]

    return x, n_sub, Wc, dest, xts, nti, NI, inv_pad, xis


def _prep_weights(inputs):
    """Quantize weight-norm'ed conv weights to fp8 main + residual."""
    v = np.asarray(inputs["weight_v"], dtype=np.float32)
    g = np.asarray(inputs["weight_g"], dtype=np.float32)
    norm = np.sqrt((v * v).sum(axis=(1, 2), keepdims=True))
    w_eff = (g * v / norm) * SW                           # [H_out, H_in, K]
    wk = np.ascontiguousarray(w_eff.transpose(2, 1, 0))   # [K, H_in, H_out]
    w8 = wk.astype(E4)
    s8 = (wk[:MTAPS] - w8[:MTAPS].astype(np.float32)).astype(E4)
    # device layout: [pair, partition, tap, slot, H_out]
    wt = np.ascontiguousarray(
        np.stack([_to_pairs(w8[k]) for k in range(K)]).transpose(1, 2, 0, 3, 4))
    st = np.ascontiguousarray(
        np.stack([_to_pairs(s8[k]) for k in range(MTAPS)]).transpose(
            1, 2, 0, 3, 4)) if MTAPS else None
    cb = (np.asarray(inputs["conv_bias"], np.float32)
          * SX * SW).astype(np.float16).reshape(1, H)
    return wt, st, cb


_PROGRAM_CACHE = {}


def _prep(inputs):
    """Pack inputs; return (program, per-core in_maps, scatter metadata)."""
    x, n_sub, Wc, dest, xts, nti, NI, inv_pad, xis = _pack(
        inputs["hidden_states"], inputs["input_lengths"])
    wt, st, cb = _prep_weights(inputs)
    gamma = np.asarray(inputs["gamma"], np.float32).reshape(H)
    beta = np.asarray(inputs["beta"], np.float32).reshape(H)
    apply_gb = not (np.allclose(gamma, 1.0) and np.allclose(beta, 0.0))

    cache_key = (n_sub, nti, apply_gb)
    nc = _PROGRAM_CACHE.get(cache_key)
    if nc is None:
        nc = _build_program(n_sub, nti, apply_gb)
        _PROGRAM_CACHE[cache_key] = nc

    in_maps = []
    for m in range(NCORES):
        im = {"xx": xts[m], "wt": wt, "cb": cb}
        if MTAPS:
            im["st"] = st
        if nti:
            im["xi"] = xis[m]
        if apply_gb:
            im["gm"] = gamma.reshape(1, H)
            im["bt"] = beta.reshape(1, H)
        in_maps.append(im)
    meta = (Wc, dest, nti, NI, inv_pad)
    return nc, in_maps, meta


def _run(inputs, trace=False):
    from concourse.bass_utils import run_bass_kernel_spmd

    nc, in_maps, meta = _prep(inputs)
    Wc, dest, nti, NI, inv_pad = meta

    res = run_bass_kernel_spmd(nc, in_maps, core_ids=list(range(NCORES)),
                               trace=trace)

    y_flat = np.empty((B * T, H), np.float32)
    for m in range(NCORES):
        yvm = np.asarray(res.results[m]["yv"]).astype(np.float32)
        dm = dest[m * Wc: (m + 1) * Wc]
        sel = dm >= 0
        y_flat[dm[sel]] = yvm[sel]
        if nti:
            yim = np.asarray(res.results[m]["yi"]).astype(
                np.float32).reshape(NI, H)
            im_idx = inv_pad[m * NI: (m + 1) * NI]
            sel = im_idx >= 0
            y_flat[im_idx[sel]] = yim[sel]

    return y_flat.reshape(B, T, H), res


def kernel(**inputs):
    out, _ = _run(inputs, trace=False)
    return out


# revision 16
# speedup vs baseline: 1.6096x; 1.0096x over previous
"""Trainium2 Bass kernel for AcousticTextEncoderLayer.

Reference computation (B=16, T=4096, H=512, K=9):
  w = weight_norm(weight_v, weight_g)            # per-out-channel scale
  x_masked = hidden_states * (t < len)           # zero beyond each length
  conv = conv1d(x_masked, w, same pad) + bias    # per-sample temporal conv
  y = where(t < len, conv, hidden_states)        # passthrough beyond length
  y = layernorm(y, gamma, beta); leaky_relu(y, 0.1)

Strategy: the per-(b,t) work splits into "valid" positions (t < len: conv +
LN) and "invalid" positions (t >= len: LN only).  The host packs all valid
positions of all samples into one zero-separated timeline (8 zero columns
between samples so the 9-tap conv never mixes samples), splits it evenly
across the 8 cores, and packs invalid rows into equal per-core blocks.
Every core runs the same program on its slice; the host scatters results
back.

Conv runs in fp8e4m3 with DoubleRow matmuls (two 128-deep contraction
tiles per instruction at 0.5 cycles/output-column — 2x the fp16 rate).
Precision is recovered by accumulating, in the same fp32 PSUM group and
at one common scale:
  x_hi.w8  (main, 18 DoubleRow matmuls per 128-position tile)
  x_lo.w8  (exact two-term fp8 split of x: kills the x-side quant error)
  x_hi.s8  (w-residual on MTAPS of the 9 taps: kills most w-side error)
LayerNorm is scale-invariant, so the fp8 scaling (x*16, w*256) and the
matching bias*4096 need no explicit rescale.  Invalid (LN-only) rows ship
as fp16 both ways; valid outputs also return as fp16 (the 2e-2 tolerance
dwarfs fp16 rounding).  LN stats via bn_stats/bn_aggr; normalize + leaky
relu fused into one scalar-engine activation with per-partition
scale/bias (Prelu, alpha=0.1).
"""

import math

import numpy as np
import ml_dtypes

B, T, H, K = 16, 4096, 512, 9
SLOPE = 0.1
EPS = 1e-5
NCORES = 8
SEG = 512          # valid-timeline columns per full segment (4 PSUM tiles)
HALO = K // 2      # 4
SEP = HALO         # zero columns between samples (taps reach <= HALO out)
NPAIR = 2          # DoubleRow chunk pairs (2 x 256 input channels)

SX = 8.0           # fp8 scale for x (LayerNorm absorbs it; small enough
                   # that scaled conv values stay well inside fp16 range)
SW = 256.0         # fp8 scale for w
MTAPS = 4          # taps with w-residual correction (of K)
E4 = ml_dtypes.float8_e4m3
WARMUP_MMS = 28    # throwaway matmuls that warm the PE clock during load


def _split_sync_waits(nc, mybir, bass_rust, max_w=1):
    """walrus in this env rejects instructions carrying more than one sync
    wait.  Prefer hoisting extra waits onto the immediately preceding
    same-engine instruction when it has spare wait slots and no sem
    updates (waiting earlier on the same engine is strictly conservative,
    and update-free hosts cannot create wait/update cycles) — this avoids
    the 71ns/inst sequencer cost of a NoOp next to every matmul in the
    weight-arrival window.  Fall back to inserted NoOps otherwise."""
    def n_waits(i):
        return len(i.sync_info.on_wait or []) if i.sync_info is not None else 0

    def can_host(i):
        if i.sync_info is not None and (i.sync_info.on_update or []):
            return False
        return isinstance(i, (mybir.InstLdweights, mybir.InstNoOp))

    for fn in nc.m.functions:
        for bb in fn.blocks:
            out = []
            changed = False
            for inst in bb.instructions:
                si = inst.sync_info
                waits = list(si.on_wait or []) if si is not None else []
                if len(waits) > max_w:
                    extra, keep = waits[:-max_w], waits[-max_w:]
                    # hoist onto the preceding instruction if possible
                    if (extra and out and out[-1].engine == inst.engine
                            and can_host(out[-1])
                            and n_waits(out[-1]) < max_w):
                        prev = out[-1]
                        room = max_w - n_waits(prev)
                        moved, extra = extra[:room], extra[room:]
                        pw = (list(prev.sync_info.on_wait or [])
                              if prev.sync_info is not None else [])
                        prev.sync_info = bass_rust.SyncInfo(
                            on_wait=pw + moved, on_update=[])
                    while extra:
                        chunk, extra = extra[:max_w], extra[max_w:]
                        nop = mybir.InstNoOp(
                            name=nc.get_next_instruction_name(), ins=[], outs=[]
                        )
                        nop.engine = inst.engine
                        nop.sync_info = bass_rust.SyncInfo(
                            on_wait=chunk, on_update=[]
                        )
                        out.append(nop)
                    inst.sync_info = bass_rust.SyncInfo(
                        on_wait=keep, on_update=list(si.on_update or [])
                    )
                    changed = True
                out.append(inst)
            if changed:
                bb.instructions[:] = out


def _build_program(n_sub, nti, apply_gb, repeat=1):
    import concourse.bass as bass
    import concourse.tile as tile
    import concourse.mybir as mybir
    import bass_rust
    from contextlib import ExitStack

    f32 = mybir.dt.float32
    f16 = mybir.dt.float16
    f8 = mybir.dt.float8e4
    DR = mybir.MatmulPerfMode.DoubleRow

    nc = bass.Bass("TRN2", target_bir_lowering=False, debug=False,
                   num_devices=NCORES)
    Wc = n_sub * 128
    # segment widths: full SEG-wide segments plus one remainder
    seg_widths = [SEG] * (Wc // SEG)
    if Wc % SEG:
        seg_widths.append(Wc % SEG)
    nseg = len(seg_widths)
    # x planes: [plane(hi/lo), pair, partition, slot, column]
    xx = nc.dram_tensor("xx", [2, NPAIR, 128, 2, Wc + 2 * HALO], f8,
                        kind="ExternalInput")
    wt = nc.dram_tensor("wt", [NPAIR, 128, K, 2, H], f8, kind="ExternalInput")
    st = None
    if MTAPS:
        st = nc.dram_tensor("st", [NPAIR, 128, MTAPS, 2, H], f8,
                            kind="ExternalInput")
    cb = nc.dram_tensor("cb", [1, H], f16, kind="ExternalInput")
    b8 = nc.dram_tensor("b8", [1, 2, H], f8, kind="ExternalInput")
    yv = nc.dram_tensor("yv", [Wc, H], f16, kind="ExternalOutput")
    xi = yi = None
    if nti:
        xi = nc.dram_tensor("xi", [nti, 128, H], f16, kind="ExternalInput")
        yi = nc.dram_tensor("yi", [nti, 128, H], f16, kind="ExternalOutput")
    gm = bt = None
    if apply_gb:
        gm = nc.dram_tensor("gm", [1, H], f32, kind="ExternalInput")
        bt = nc.dram_tensor("bt", [1, H], f32, kind="ExternalInput")

    AF = mybir.ActivationFunctionType
    OP = mybir.AluOpType

    with tile.TileContext(nc) as tc, ExitStack() as ctx:
        consts = ctx.enter_context(tc.tile_pool(name="consts", bufs=1))
        xpool = ctx.enter_context(tc.tile_pool(name="xpool", bufs=8))
        ipool = ctx.enter_context(tc.tile_pool(name="ipool", bufs=4))
        psum = ctx.enter_context(tc.tile_pool(name="psum", bufs=8, space="PSUM"))
        ypool = ctx.enter_context(tc.tile_pool(name="ypool", bufs=6))
        opool = ctx.enter_context(tc.tile_pool(name="opool", bufs=6))
        spool = ctx.enter_context(tc.tile_pool(name="spool", bufs=8))

        def seg_strips(seg_start, sw, planes=(0, 1), into=None):
            # 4 strips per segment: (plane hi/lo) x (chunk pair), each
            # [128, 2(slot), sw + 2*HALO] fp8.  The slot-dim stride of a
            # DoubleRow ldweights AP must be a multiple of 16 bytes, so the
            # tile width is padded up (the pad columns are never read).
            w_used = sw + 2 * HALO
            w_pad = (w_used + 15) // 16 * 16
            strips = into if into is not None else [None, None]
            for plane in planes:
                row = []
                for pr in range(NPAIR):
                    strip = xpool.tile([128, 2, w_pad], f8,
                                       tag=f"strip{plane}_{pr}")
                    nc.sync.dma_start(
                        out=strip[:, :, 0:w_used],
                        in_=xx[plane, pr, :, :,
                               seg_start: seg_start + w_used])
                    row.append(strip)
                strips[plane] = row
            return strips

        # Startup DMA order matches first-tile consumption order (hi-plane
        # strips, w8 blocks, lo-plane strips, w-residual blocks) — the cost
        # model serializes DMA through one HWDGE device, so order is what
        # hides the load latency behind the PE warmup.  Weights come as one
        # DMA per chunk pair so the PE is not gated on dozens of small
        # transfers.
        strips0 = seg_strips(0, seg_widths[0], planes=(0,))
        wblocks = []
        for pr in range(NPAIR):
            wbl = consts.tile([128, K, 2, H], f8, tag=f"wb{pr}")
            nc.sync.dma_start(out=wbl, in_=wt[pr, :, :, :, :])
            wblocks.append(wbl)
        seg_strips(0, seg_widths[0], planes=(1,), into=strips0)
        sblocks = []
        for pr in range(NPAIR):
            if not MTAPS:
                break
            sbl = consts.tile([128, MTAPS, 2, H], f8, tag=f"sb{pr}")
            nc.sync.dma_start(out=sbl, in_=st[pr, :, :, :, :])
            sblocks.append(sbl)
        wtiles = [[wblocks[pr][:, k, :, :] for k in range(K)]
                  for pr in range(NPAIR)]
        stiles = [[sblocks[pr][:, k, :, :] for k in range(MTAPS)]
                  for pr in range(NPAIR)] if MTAPS else []
        bias_b = consts.tile([128, H], f16, tag="bias_b")
        nc.sync.dma_start(out=bias_b, in_=cb.ap().to_broadcast((128, H)))
        # bias as a rank-1 fp8 matmul operand (used on the final tile only,
        # to shorten the end-of-program LN chain): one-hot stationary picks
        # partition 0; the two DoubleRow slots carry a hi/lo split of the
        # scaled bias.
        oneh = consts.tile([128, 2, 128], f8, tag="oneh")
        nc.vector.memset(oneh, 0.0)
        nc.vector.memset(oneh[0:1, :, :], 1.0)
        bias8 = consts.tile([128, 2, H], f8, tag="bias8")
        nc.vector.memset(bias8, 0.0)
        nc.sync.dma_start(out=bias8[0:1, :, :], in_=b8.ap())
        gm_b = bt_b = None
        if apply_gb:
            gm_b = consts.tile([128, H], f32, tag="gm_b")
            nc.sync.dma_start(out=gm_b, in_=gm.ap().to_broadcast((128, H)))
            bt_b = consts.tile([128, H], f32, tag="bt_b")
            nc.sync.dma_start(out=bt_b, in_=bt.ap().to_broadcast((128, H)))
        eps_t = consts.tile([128, 1], f32, tag="eps")
        nc.vector.memset(eps_t, EPS)

        # Warm up the PE clock (HAM gate: 1.2 -> 2.4 GHz after ~3us of
        # sustained activity) with throwaway matmuls on a zeroed tile while
        # the first strips/weights are still in flight.  Results go to a
        # scratch PSUM bank nobody reads.
        if WARMUP_MMS:
            wu_src = consts.tile([128, 2, 128], f8, tag="wu_src")
            nc.vector.memset(wu_src, 0.0)
            wu_mov = consts.tile([128, 2, H], f8, tag="wu_mov")
            nc.vector.memset(wu_mov, 0.0)
            wu_ps = psum.tile([128, H], f32, tag="ps")
            for i in range(WARMUP_MMS):
                nc.tensor.matmul(wu_ps, wu_src, wu_mov,
                                 start=(i == 0), stop=(i == WARMUP_MMS - 1),
                                 perf_mode=DR)

        def ln_lrelu(src, dst):
            # LayerNorm over the free dim + leaky relu, into dst.
            stt = spool.tile([128, 6], f32, tag="st")
            nc.vector.bn_stats(out=stt, in_=src)
            mv = spool.tile([128, 2], f32, tag="mv")
            nc.vector.bn_aggr(out=mv, in_=stt)
            sd = spool.tile([128, 1], f32, tag="sd")
            nc.scalar.activation(out=sd, in_=mv[:, 1:2], func=AF.Sqrt,
                                 bias=eps_t, scale=1.0)
            rstd = spool.tile([128, 1], f32, tag="rstd")
            nc.vector.reciprocal(out=rstd, in_=sd)
            nms = spool.tile([128, 1], f32, tag="nms")
            nc.vector.tensor_scalar(out=nms, in0=mv[:, 0:1], scalar1=rstd,
                                    scalar2=-1.0, op0=OP.mult, op1=OP.mult)
            if not apply_gb:
                nc.scalar.activation(out=dst, in_=src, func=AF.Prelu,
                                     bias=nms, scale=rstd, alpha=SLOPE)
            else:
                tmp = spool.tile([128, H], f32, tag="gbtmp")
                nc.scalar.activation(out=tmp, in_=src, func=AF.Identity,
                                     bias=nms, scale=rstd)
                nc.vector.tensor_mul(out=tmp, in0=tmp, in1=gm_b)
                nc.vector.tensor_add(out=tmp, in0=tmp, in1=bt_b)
                nc.scalar.activation(out=dst, in_=tmp, func=AF.Prelu,
                                     alpha=SLOPE)

        def invalid_tile(it):
            xti = ipool.tile([128, H], f16, tag="xi")
            nc.sync.dma_start(out=xti, in_=xi[it, :, :])
            oi = opool.tile([128, H], f16, tag="oi")
            ln_lrelu(xti, oi)
            nc.sync.dma_start(out=yi[it, :, :], in_=oi)

        # Interleave invalid (LN-only) tiles among valid segments so the
        # vector/scalar engines fill PE-wait gaps.  (repeat>1 re-runs the
        # whole body with identical I/O — used only for differential
        # wall-clock timing, never for the graded kernel.)
        for _rep in range(repeat):
          done_inv = 0
          seg_start = 0
          for s, sw in enumerate(seg_widths):
            if s == 0 and _rep == 0:
                strips = strips0
            else:
                strips = seg_strips(seg_start, sw)
            for sub in range(sw // 128):
                final = (s == nseg - 1 and sub == sw // 128 - 1)
                ps = psum.tile([128, H], f32, tag="ps")
                # one fp32 PSUM accumulation group per tile:
                #   x_hi.w8 (all taps) + x_lo.w8 (all taps) + x_hi.s8
                seq = []
                for pr in range(NPAIR):
                    for k in range(K):
                        seq.append((strips[0][pr], k, wtiles[pr][k]))
                for pr in range(NPAIR):
                    for k in range(K):
                        seq.append((strips[1][pr], k, wtiles[pr][k]))
                for pr in range(NPAIR):
                    for k in range(MTAPS):
                        seq.append((strips[0][pr], k, stiles[pr][k]))
                if final:
                    # fold the bias add into the PE so the trailing LN chain
                    # (the only one not hidden under matmuls) is shorter
                    seq.append((None, 0, bias8))
                last = len(seq) - 1
                for i, (strip, k, wti) in enumerate(seq):
                    lhsT = (oneh if strip is None else
                            strip[:, :, sub * 128 + k: sub * 128 + k + 128])
                    nc.tensor.matmul(
                        ps, lhsT, wti,
                        start=(i == 0),
                        stop=(i == last),
                        perf_mode=DR,
                    )
                o = opool.tile([128, H], f16, tag="o")
                if final:
                    ln_lrelu(ps, o)
                else:
                    y = ypool.tile([128, H], f16, tag="y")
                    nc.vector.tensor_add(out=y, in0=ps, in1=bias_b)
                    ln_lrelu(y, o)
                row0 = seg_start + sub * 128
                nc.sync.dma_start(out=yv[row0: row0 + 128, :], in_=o)
            seg_start += sw
            inv_target = (0 if s < 2 and s < nseg - 2 else
                          min(nti, s * nti // max(1, nseg - 3)))
            while done_inv < inv_target:
                invalid_tile(done_inv)
                done_inv += 1
        while done_inv < nti:
            invalid_tile(done_inv)
            done_inv += 1

    _split_sync_waits(nc, mybir, bass_rust)
    return nc


def _to_pairs(a):
    """[512, ...cols] -> [pair, 128, slot, ...cols] with
    channel = pair*256 + slot*128 + partition."""
    s = a.shape[1:]
    return np.ascontiguousarray(
        a.reshape(NPAIR, 2, 128, *s).transpose(0, 2, 1, 3))


def _pack(hidden_states, input_lengths):
    """Build per-core packed fp8 inputs + scatter indices."""
    x = np.ascontiguousarray(np.asarray(hidden_states, dtype=np.float32))
    lens = np.asarray(input_lengths).astype(np.int64).clip(0, T)

    starts = np.zeros(B, np.int64)
    col = 0
    for b in range(B):
        starts[b] = col
        col += int(lens[b]) + SEP
    Wt = col
    n_sub = max(1, math.ceil(math.ceil(Wt / NCORES) / 128))
    Wc = n_sub * 128
    Wtot = NCORES * Wc

    XTL = np.zeros((H, HALO + Wtot + HALO), np.float32)
    dest = np.full(Wtot, -1, np.int64)
    for b in range(B):
        L = int(lens[b])
        s0 = int(starts[b])
        XTL[:, HALO + s0: HALO + s0 + L] = x[b, :L, :].T
        dest[s0: s0 + L] = b * T + np.arange(L, dtype=np.int64)

    XTL *= SX
    hi8 = XTL.astype(E4)
    lo8 = (XTL - hi8.astype(np.float32)).astype(E4)

    xts = []
    for m in range(NCORES):
        sl_h = _to_pairs(hi8[:, m * Wc: m * Wc + Wc + 2 * HALO])
        sl_l = _to_pairs(lo8[:, m * Wc: m * Wc + Wc + 2 * HALO])
        xts.append(np.ascontiguousarray(np.stack([sl_h, sl_l])))

    # invalid rows
    inv_mask = (np.arange(T)[None, :] >= lens[:, None]).ravel()
    inv_idx = np.nonzero(inv_mask)[0]
    I = len(inv_idx)
    nti = math.ceil(I / (NCORES * 128)) if I else 0
    NI = nti * 128
    xis = None
    inv_pad = None
    if nti:
        x_flat = x.reshape(B * T, H)
        xi_all = np.zeros((NCORES * NI, H), np.float16)
        xi_all[:I] = x_flat[inv_idx]
        inv_pad = np.full(NCORES * NI, -1, np.int64)
        inv_pad[:I] = inv_idx
        xis = [np.ascontiguousarray(
            xi_all[m * NI: (m + 1) * NI].reshape(nti, 128, H))
            for m in range(NCORES)]

    return x, n_sub, Wc, dest, xts, nti, NI, inv_pad, xis


def _prep_weights(inputs):
    """Quantize weight-norm'ed conv weights to fp8 main + residual."""
    v = np.asarray(inputs["weight_v"], dtype=np.float32)
    g = np.asarray(inputs["weight_g"], dtype=np.float32)
    norm = np.sqrt((v * v).sum(axis=(1, 2), keepdims=True))
    w_eff = (g * v / norm) * SW                           # [H_out, H_in, K]
    wk = np.ascontiguousarray(w_eff.transpose(2, 1, 0))   # [K, H_in, H_out]
    w8 = wk.astype(E4)
    s8 = (wk[:MTAPS] - w8[:MTAPS].astype(np.float32)).astype(E4)
    # device layout: [pair, partition, tap, slot, H_out]
    wt = np.ascontiguousarray(
        np.stack([_to_pairs(w8[k]) for k in range(K)]).transpose(1, 2, 0, 3, 4))
    st = np.ascontiguousarray(
        np.stack([_to_pairs(s8[k]) for k in range(MTAPS)]).transpose(
            1, 2, 0, 3, 4)) if MTAPS else None
    b_s = np.asarray(inputs["conv_bias"], np.float32) * SX * SW
    cb = b_s.astype(np.float16).reshape(1, H)
    # rank-1 fp8 bias for the final-tile fold: hi/lo split keeps the
    # representation error at the fp8-of-residual level
    b_hi = (b_s * 0.5).astype(E4)
    b_lo = (b_s - b_hi.astype(np.float32)).astype(E4)
    b8 = np.ascontiguousarray(
        np.stack([b_hi, b_lo]).reshape(1, 2, H))
    return wt, st, cb, b8


_PROGRAM_CACHE = {}


def _prep(inputs):
    """Pack inputs; return (program, per-core in_maps, scatter metadata)."""
    x, n_sub, Wc, dest, xts, nti, NI, inv_pad, xis = _pack(
        inputs["hidden_states"], inputs["input_lengths"])
    wt, st, cb, b8 = _prep_weights(inputs)
    gamma = np.asarray(inputs["gamma"], np.float32).reshape(H)
    beta = np.asarray(inputs["beta"], np.float32).reshape(H)
    apply_gb = not (np.allclose(gamma, 1.0) and np.allclose(beta, 0.0))

    cache_key = (n_sub, nti, apply_gb)
    nc = _PROGRAM_CACHE.get(cache_key)
    if nc is None:
        nc = _build_program(n_sub, nti, apply_gb)
        _PROGRAM_CACHE[cache_key] = nc

    in_maps = []
    for m in range(NCORES):
        im = {"xx": xts[m], "wt": wt, "cb": cb, "b8": b8}
        if MTAPS:
            im["st"] = st
        if nti:
            im["xi"] = xis[m]
        if apply_gb:
            im["gm"] = gamma.reshape(1, H)
            im["bt"] = beta.reshape(1, H)
        in_maps.append(im)
    meta = (Wc, dest, nti, NI, inv_pad)
    return nc, in_maps, meta


def _run(inputs, trace=False):
    from concourse.bass_utils import run_bass_kernel_spmd

    nc, in_maps, meta = _prep(inputs)
    Wc, dest, nti, NI, inv_pad = meta

    res = run_bass_kernel_spmd(nc, in_maps, core_ids=list(range(NCORES)),
                               trace=trace)

    y_flat = np.empty((B * T, H), np.float32)
    for m in range(NCORES):
        yvm = np.asarray(res.results[m]["yv"]).astype(np.float32)
        dm = dest[m * Wc: (m + 1) * Wc]
        sel = dm >= 0
        y_flat[dm[sel]] = yvm[sel]
        if nti:
            yim = np.asarray(res.results[m]["yi"]).astype(
                np.float32).reshape(NI, H)
            im_idx = inv_pad[m * NI: (m + 1) * NI]
            sel = im_idx >= 0
            y_flat[im_idx[sel]] = yim[sel]

    return y_flat.reshape(B, T, H), res


def kernel(**inputs):
    out, _ = _run(inputs, trace=False)
    return out


# revision 23
# speedup vs baseline: 1.6391x; 1.0183x over previous
"""Trainium2 Bass kernel for AcousticTextEncoderLayer.

Reference computation (B=16, T=4096, H=512, K=9):
  w = weight_norm(weight_v, weight_g)            # per-out-channel scale
  x_masked = hidden_states * (t < len)           # zero beyond each length
  conv = conv1d(x_masked, w, same pad) + bias    # per-sample temporal conv
  y = where(t < len, conv, hidden_states)        # passthrough beyond length
  y = layernorm(y, gamma, beta); leaky_relu(y, 0.1)

Strategy: the per-(b,t) work splits into "valid" positions (t < len: conv +
LN) and "invalid" positions (t >= len: LN only).  The host packs all valid
positions of all samples into one zero-separated timeline (8 zero columns
between samples so the 9-tap conv never mixes samples), splits it evenly
across the 8 cores, and packs invalid rows into equal per-core blocks.
Every core runs the same program on its slice; the host scatters results
back.

Conv runs in fp8e4m3 with DoubleRow matmuls (two 128-deep contraction
tiles per instruction at 0.5 cycles/output-column — 2x the fp16 rate).
Precision is recovered by accumulating, in the same fp32 PSUM group and
at one common scale:
  x_hi.w8  (main, 18 DoubleRow matmuls per 128-position tile)
  x_lo.w8  (exact two-term fp8 split of x: kills the x-side quant error)
  x_hi.s8  (w-residual on MTAPS of the 9 taps: kills most w-side error)
LayerNorm is scale-invariant, so the fp8 scaling (x*16, w*256) and the
matching bias*4096 need no explicit rescale.  Invalid (LN-only) rows ship
as fp16 both ways; valid outputs also return as fp16 (the 2e-2 tolerance
dwarfs fp16 rounding).  LN stats via bn_stats/bn_aggr; normalize + leaky
relu fused into one scalar-engine activation with per-partition
scale/bias (Prelu, alpha=0.1).
"""

import math

import numpy as np
import ml_dtypes

B, T, H, K = 16, 4096, 512, 9
SLOPE = 0.1
EPS = 1e-5
NCORES = 8
SEG = 512          # valid-timeline columns per full segment (4 PSUM tiles)
HALO = K // 2      # 4
SEP = HALO         # zero columns between samples (taps reach <= HALO out)
NPAIR = 2          # DoubleRow chunk pairs (2 x 256 input channels)

SX = 8.0           # fp8 scale for x (LayerNorm absorbs it; small enough
                   # that scaled conv values stay well inside fp16 range)
SW = 256.0         # fp8 scale for w
MTAPS = 4          # taps with w-residual correction (of K)
W_RES_UNITS = 7    # (pair, tap) w-residual matmuls actually emitted
                   # (pr-major order; 7 of the 2*MTAPS=8 possible)
E4 = ml_dtypes.float8_e4m3
WARMUP_MMS = 28    # throwaway matmuls that warm the PE clock during load


def _split_sync_waits(nc, mybir, bass_rust, max_w=1):
    """walrus in this env rejects instructions carrying more than one sync
    wait.  Prefer hoisting extra waits onto the immediately preceding
    same-engine instruction when it has spare wait slots and no sem
    updates (waiting earlier on the same engine is strictly conservative,
    and update-free hosts cannot create wait/update cycles) — this avoids
    the 71ns/inst sequencer cost of a NoOp next to every matmul in the
    weight-arrival window.  Fall back to inserted NoOps otherwise."""
    def n_waits(i):
        return len(i.sync_info.on_wait or []) if i.sync_info is not None else 0

    def can_host(i):
        if i.sync_info is not None and (i.sync_info.on_update or []):
            return False
        return isinstance(i, (mybir.InstLdweights, mybir.InstNoOp))

    for fn in nc.m.functions:
        for bb in fn.blocks:
            out = []
            changed = False
            for inst in bb.instructions:
                si = inst.sync_info
                waits = list(si.on_wait or []) if si is not None else []
                if len(waits) > max_w:
                    extra, keep = waits[:-max_w], waits[-max_w:]
                    # hoist onto the preceding instruction if possible
                    if (extra and out and out[-1].engine == inst.engine
                            and can_host(out[-1])
                            and n_waits(out[-1]) < max_w):
                        prev = out[-1]
                        room = max_w - n_waits(prev)
                        moved, extra = extra[:room], extra[room:]
                        pw = (list(prev.sync_info.on_wait or [])
                              if prev.sync_info is not None else [])
                        prev.sync_info = bass_rust.SyncInfo(
                            on_wait=pw + moved, on_update=[])
                    while extra:
                        chunk, extra = extra[:max_w], extra[max_w:]
                        nop = mybir.InstNoOp(
                            name=nc.get_next_instruction_name(), ins=[], outs=[]
                        )
                        nop.engine = inst.engine
                        nop.sync_info = bass_rust.SyncInfo(
                            on_wait=chunk, on_update=[]
                        )
                        out.append(nop)
                    inst.sync_info = bass_rust.SyncInfo(
                        on_wait=keep, on_update=list(si.on_update or [])
                    )
                    changed = True
                out.append(inst)
            if changed:
                bb.instructions[:] = out


def _build_program(n_sub, nti, apply_gb, repeat=1):
    import concourse.bass as bass
    import concourse.tile as tile
    import concourse.mybir as mybir
    import bass_rust
    from contextlib import ExitStack

    f32 = mybir.dt.float32
    f16 = mybir.dt.float16
    f8 = mybir.dt.float8e4
    DR = mybir.MatmulPerfMode.DoubleRow

    nc = bass.Bass("TRN2", target_bir_lowering=False, debug=False,
                   num_devices=NCORES)
    Wc = n_sub * 128
    # segment widths: full SEG-wide segments plus one remainder
    seg_widths = [SEG] * (Wc // SEG)
    if Wc % SEG:
        seg_widths.append(Wc % SEG)
    nseg = len(seg_widths)
    # x planes: [plane(hi/lo), pair, partition, slot, column]
    xx = nc.dram_tensor("xx", [2, NPAIR, 128, 2, Wc + 2 * HALO], f8,
                        kind="ExternalInput")
    wt = nc.dram_tensor("wt", [NPAIR, 128, K, 2, H], f8, kind="ExternalInput")
    st = None
    if MTAPS:
        st = nc.dram_tensor("st", [NPAIR, 128, MTAPS, 2, H], f8,
                            kind="ExternalInput")
    cb = nc.dram_tensor("cb", [1, H], f16, kind="ExternalInput")
    b8 = nc.dram_tensor("b8", [1, 2, H], f8, kind="ExternalInput")
    yv = nc.dram_tensor("yv", [Wc, H], f16, kind="ExternalOutput")
    xi = yi = None
    if nti:
        xi = nc.dram_tensor("xi", [nti, 128, H], f16, kind="ExternalInput")
        yi = nc.dram_tensor("yi", [nti, 128, H], f16, kind="ExternalOutput")
    gm = bt = None
    if apply_gb:
        gm = nc.dram_tensor("gm", [1, H], f32, kind="ExternalInput")
        bt = nc.dram_tensor("bt", [1, H], f32, kind="ExternalInput")

    AF = mybir.ActivationFunctionType
    OP = mybir.AluOpType

    with tile.TileContext(nc) as tc, ExitStack() as ctx:
        consts = ctx.enter_context(tc.tile_pool(name="consts", bufs=1))
        xpool = ctx.enter_context(tc.tile_pool(name="xpool", bufs=8))
        ipool = ctx.enter_context(tc.tile_pool(name="ipool", bufs=4))
        psum = ctx.enter_context(tc.tile_pool(name="psum", bufs=8, space="PSUM"))
        ypool = ctx.enter_context(tc.tile_pool(name="ypool", bufs=6))
        opool = ctx.enter_context(tc.tile_pool(name="opool", bufs=6))
        spool = ctx.enter_context(tc.tile_pool(name="spool", bufs=8))

        def seg_strips(seg_start, sw, planes=(0, 1), into=None):
            # 4 strips per segment: (plane hi/lo) x (chunk pair), each
            # [128, 2(slot), sw + 2*HALO] fp8.  The slot-dim stride of a
            # DoubleRow ldweights AP must be a multiple of 16 bytes, so the
            # tile width is padded up (the pad columns are never read).
            w_used = sw + 2 * HALO
            w_pad = (w_used + 15) // 16 * 16
            strips = into if into is not None else [None, None]
            for plane in planes:
                row = []
                for pr in range(NPAIR):
                    strip = xpool.tile([128, 2, w_pad], f8,
                                       tag=f"strip{plane}_{pr}")
                    nc.sync.dma_start(
                        out=strip[:, :, 0:w_used],
                        in_=xx[plane, pr, :, :,
                               seg_start: seg_start + w_used])
                    row.append(strip)
                strips[plane] = row
            return strips

        # Startup DMA order matches first-tile consumption order (hi-plane
        # strips, w8 blocks, lo-plane strips, w-residual blocks) — the cost
        # model serializes DMA through one HWDGE device, so order is what
        # hides the load latency behind the PE warmup.  Weights come as one
        # DMA per chunk pair so the PE is not gated on dozens of small
        # transfers.
        strips0 = seg_strips(0, seg_widths[0], planes=(0,))
        wblocks = []
        for pr in range(NPAIR):
            wbl = consts.tile([128, K, 2, H], f8, tag=f"wb{pr}")
            nc.sync.dma_start(out=wbl, in_=wt[pr, :, :, :, :])
            wblocks.append(wbl)
        seg_strips(0, seg_widths[0], planes=(1,), into=strips0)
        # per-pair w-residual tap counts (pr-major split of W_RES_UNITS)
        staps = [min(MTAPS, max(0, W_RES_UNITS - pr * MTAPS))
                 for pr in range(NPAIR)]
        sblocks = []
        for pr in range(NPAIR):
            if not staps[pr]:
                sblocks.append(None)
                continue
            sbl = consts.tile([128, staps[pr], 2, H], f8, tag=f"sb{pr}")
            nc.sync.dma_start(out=sbl, in_=st[pr, :, 0:staps[pr], :, :])
            sblocks.append(sbl)
        wtiles = [[wblocks[pr][:, k, :, :] for k in range(K)]
                  for pr in range(NPAIR)]
        stiles = [[sblocks[pr][:, k, :, :] for k in range(staps[pr])]
                  if sblocks[pr] is not None else []
                  for pr in range(NPAIR)]
        bias_b = consts.tile([128, H], f16, tag="bias_b")
        nc.sync.dma_start(out=bias_b, in_=cb.ap().to_broadcast((128, H)))
        # bias as a rank-1 fp8 matmul operand (used on the final tile only,
        # to shorten the end-of-program LN chain): one-hot stationary picks
        # partition 0; the two DoubleRow slots carry a hi/lo split of the
        # scaled bias.
        oneh = consts.tile([128, 2, 128], f8, tag="oneh")
        nc.vector.memset(oneh, 0.0)
        nc.vector.memset(oneh[0:1, :, :], 1.0)
        bias8 = consts.tile([128, 2, H], f8, tag="bias8")
        nc.vector.memset(bias8, 0.0)
        nc.sync.dma_start(out=bias8[0:1, :, :], in_=b8.ap())
        gm_b = bt_b = None
        if apply_gb:
            gm_b = consts.tile([128, H], f32, tag="gm_b")
            nc.sync.dma_start(out=gm_b, in_=gm.ap().to_broadcast((128, H)))
            bt_b = consts.tile([128, H], f32, tag="bt_b")
            nc.sync.dma_start(out=bt_b, in_=bt.ap().to_broadcast((128, H)))
        eps_t = consts.tile([128, 1], f32, tag="eps")
        nc.vector.memset(eps_t, EPS)

        # Warm up the PE clock (HAM gate: 1.2 -> 2.4 GHz after ~3us of
        # sustained activity) with throwaway matmuls on a zeroed tile while
        # the first strips/weights are still in flight.  Results go to a
        # scratch PSUM bank nobody reads.
        if WARMUP_MMS:
            wu_src = consts.tile([128, 2, 128], f8, tag="wu_src")
            nc.vector.memset(wu_src, 0.0)
            wu_mov = consts.tile([128, 2, H], f8, tag="wu_mov")
            nc.vector.memset(wu_mov, 0.0)
            wu_ps = psum.tile([128, H], f32, tag="ps")
            for i in range(WARMUP_MMS):
                nc.tensor.matmul(wu_ps, wu_src, wu_mov,
                                 start=(i == 0), stop=(i == WARMUP_MMS - 1),
                                 perf_mode=DR)

        def ln_stats(src):
            # per-partition LayerNorm stats: returns (-mean*rstd, rstd)
            stt = spool.tile([128, 6], f32, tag="st")
            nc.vector.bn_stats(out=stt, in_=src)
            mv = spool.tile([128, 2], f32, tag="mv")
            nc.vector.bn_aggr(out=mv, in_=stt)
            sd = spool.tile([128, 1], f32, tag="sd")
            nc.scalar.activation(out=sd, in_=mv[:, 1:2], func=AF.Sqrt,
                                 bias=eps_t, scale=1.0)
            rstd = spool.tile([128, 1], f32, tag="rstd")
            nc.vector.reciprocal(out=rstd, in_=sd)
            nms = spool.tile([128, 1], f32, tag="nms")
            nc.vector.tensor_scalar(out=nms, in0=mv[:, 0:1], scalar1=rstd,
                                    scalar2=-1.0, op0=OP.mult, op1=OP.mult)
            return nms, rstd

        def ln_lrelu(src, dst):
            # LayerNorm over the free dim + leaky relu, into dst.
            nms, rstd = ln_stats(src)
            if not apply_gb:
                nc.scalar.activation(out=dst, in_=src, func=AF.Prelu,
                                     bias=nms, scale=rstd, alpha=SLOPE)
            else:
                tmp = spool.tile([128, H], f32, tag="gbtmp")
                nc.scalar.activation(out=tmp, in_=src, func=AF.Identity,
                                     bias=nms, scale=rstd)
                nc.vector.tensor_mul(out=tmp, in0=tmp, in1=gm_b)
                nc.vector.tensor_add(out=tmp, in0=tmp, in1=bt_b)
                nc.scalar.activation(out=dst, in_=tmp, func=AF.Prelu,
                                     alpha=SLOPE)

        def invalid_tile(it):
            xti = ipool.tile([128, H], f16, tag="xi")
            nc.sync.dma_start(out=xti, in_=xi[it, :, :])
            oi = opool.tile([128, H], f16, tag="oi")
            ln_lrelu(xti, oi)
            nc.sync.dma_start(out=yi[it, :, :], in_=oi)

        # Interleave invalid (LN-only) tiles among valid segments so the
        # vector/scalar engines fill PE-wait gaps.  (repeat>1 re-runs the
        # whole body with identical I/O — used only for differential
        # wall-clock timing, never for the graded kernel.)
        for _rep in range(repeat):
          done_inv = 0
          seg_start = 0
          for s, sw in enumerate(seg_widths):
            if s == 0 and _rep == 0:
                strips = strips0
            else:
                strips = seg_strips(seg_start, sw)
            for sub in range(sw // 128):
                final = (s == nseg - 1 and sub == sw // 128 - 1)
                ps = psum.tile([128, H], f32, tag="ps")
                # one fp32 PSUM accumulation group per tile:
                #   x_hi.w8 (all taps) + x_lo.w8 (all taps) + x_hi.s8
                seq = []
                for pr in range(NPAIR):
                    for k in range(K):
                        seq.append((strips[0][pr], k, wtiles[pr][k]))
                for pr in range(NPAIR):
                    for k in range(K):
                        seq.append((strips[1][pr], k, wtiles[pr][k]))
                for pr in range(NPAIR):
                    for k in range(staps[pr]):
                        seq.append((strips[0][pr], k, stiles[pr][k]))
                if final:
                    # fold the bias add into the PE so the trailing LN chain
                    # (the only one not hidden under matmuls) is shorter
                    seq.append((None, 0, bias8))
                last = len(seq) - 1
                for i, (strip, k, wti) in enumerate(seq):
                    lhsT = (oneh if strip is None else
                            strip[:, :, sub * 128 + k: sub * 128 + k + 128])
                    nc.tensor.matmul(
                        ps, lhsT, wti,
                        start=(i == 0),
                        stop=(i == last),
                        perf_mode=DR,
                    )
                o = opool.tile([128, H], f16, tag="o")
                row0 = seg_start + sub * 128
                if final:
                    ln_lrelu(ps, o)
                else:
                    y = ypool.tile([128, H], f16, tag="y")
                    nc.vector.tensor_add(out=y, in0=ps, in1=bias_b)
                    ln_lrelu(y, o)
                nc.sync.dma_start(out=yv[row0: row0 + 128, :], in_=o)
            seg_start += sw
            inv_target = (0 if s < 2 and s < nseg - 2 else
                          min(nti, s * nti // max(1, nseg - 3)))
            while done_inv < inv_target:
                invalid_tile(done_inv)
                done_inv += 1
        while done_inv < nti:
            invalid_tile(done_inv)
            done_inv += 1

    _split_sync_waits(nc, mybir, bass_rust)
    return nc


def _to_pairs(a):
    """[512, ...cols] -> [pair, 128, slot, ...cols] with
    channel = pair*256 + slot*128 + partition."""
    s = a.shape[1:]
    return np.ascontiguousarray(
        a.reshape(NPAIR, 2, 128, *s).transpose(0, 2, 1, 3))


def _pack(hidden_states, input_lengths):
    """Build per-core packed fp8 inputs + scatter indices."""
    x = np.ascontiguousarray(np.asarray(hidden_states, dtype=np.float32))
    lens = np.asarray(input_lengths).astype(np.int64).clip(0, T)

    starts = np.zeros(B, np.int64)
    col = 0
    for b in range(B):
        starts[b] = col
        col += int(lens[b]) + SEP
    Wt = col
    n_sub = max(1, math.ceil(math.ceil(Wt / NCORES) / 128))
    Wc = n_sub * 128
    Wtot = NCORES * Wc

    XTL = np.zeros((H, HALO + Wtot + HALO), np.float32)
    dest = np.full(Wtot, -1, np.int64)
    for b in range(B):
        L = int(lens[b])
        s0 = int(starts[b])
        XTL[:, HALO + s0: HALO + s0 + L] = x[b, :L, :].T
        dest[s0: s0 + L] = b * T + np.arange(L, dtype=np.int64)

    XTL *= SX
    hi8 = XTL.astype(E4)
    lo8 = (XTL - hi8.astype(np.float32)).astype(E4)

    xts = []
    for m in range(NCORES):
        sl_h = _to_pairs(hi8[:, m * Wc: m * Wc + Wc + 2 * HALO])
        sl_l = _to_pairs(lo8[:, m * Wc: m * Wc + Wc + 2 * HALO])
        xts.append(np.ascontiguousarray(np.stack([sl_h, sl_l])))

    # invalid rows
    inv_mask = (np.arange(T)[None, :] >= lens[:, None]).ravel()
    inv_idx = np.nonzero(inv_mask)[0]
    I = len(inv_idx)
    nti = math.ceil(I / (NCORES * 128)) if I else 0
    NI = nti * 128
    xis = None
    inv_pad = None
    if nti:
        x_flat = x.reshape(B * T, H)
        xi_all = np.zeros((NCORES * NI, H), np.float16)
        xi_all[:I] = x_flat[inv_idx]
        inv_pad = np.full(NCORES * NI, -1, np.int64)
        inv_pad[:I] = inv_idx
        xis = [np.ascontiguousarray(
            xi_all[m * NI: (m + 1) * NI].reshape(nti, 128, H))
            for m in range(NCORES)]

    return x, n_sub, Wc, dest, xts, nti, NI, inv_pad, xis


def _prep_weights(inputs):
    """Quantize weight-norm'ed conv weights to fp8 main + residual."""
    v = np.asarray(inputs["weight_v"], dtype=np.float32)
    g = np.asarray(inputs["weight_g"], dtype=np.float32)
    norm = np.sqrt((v * v).sum(axis=(1, 2), keepdims=True))
    w_eff = (g * v / norm) * SW                           # [H_out, H_in, K]
    wk = np.ascontiguousarray(w_eff.transpose(2, 1, 0))   # [K, H_in, H_out]
    w8 = wk.astype(E4)
    s8 = (wk[:MTAPS] - w8[:MTAPS].astype(np.float32)).astype(E4)
    # device layout: [pair, partition, tap, slot, H_out]
    wt = np.ascontiguousarray(
        np.stack([_to_pairs(w8[k]) for k in range(K)]).transpose(1, 2, 0, 3, 4))
    st = np.ascontiguousarray(
        np.stack([_to_pairs(s8[k]) for k in range(MTAPS)]).transpose(
            1, 2, 0, 3, 4)) if MTAPS else None
    b_s = np.asarray(inputs["conv_bias"], np.float32) * SX * SW
    cb = b_s.astype(np.float16).reshape(1, H)
    # rank-1 fp8 bias for the final-tile fold: hi/lo split keeps the
    # representation error at the fp8-of-residual level
    b_hi = (b_s * 0.5).astype(E4)
    b_lo = (b_s - b_hi.astype(np.float32)).astype(E4)
    b8 = np.ascontiguousarray(
        np.stack([b_hi, b_lo]).reshape(1, 2, H))
    return wt, st, cb, b8


_PROGRAM_CACHE = {}


def _prep(inputs):
    """Pack inputs; return (program, per-core in_maps, scatter metadata)."""
    x, n_sub, Wc, dest, xts, nti, NI, inv_pad, xis = _pack(
        inputs["hidden_states"], inputs["input_lengths"])
    wt, st, cb, b8 = _prep_weights(inputs)
    gamma = np.asarray(inputs["gamma"], np.float32).reshape(H)
    beta = np.asarray(inputs["beta"], np.float32).reshape(H)
    apply_gb = not (np.allclose(gamma, 1.0) and np.allclose(beta, 0.0))

    cache_key = (n_sub, nti, apply_gb)
    nc = _PROGRAM_CACHE.get(cache_key)
    if nc is None:
        nc = _build_program(n_sub, nti, apply_gb)
        _PROGRAM_CACHE[cache_key] = nc

    in_maps = []
    for m in range(NCORES):
        im = {"xx": xts[m], "wt": wt, "cb": cb, "b8": b8}
        if MTAPS:
            im["st"] = st
        if nti:
            im["xi"] = xis[m]
        if apply_gb:
            im["gm"] = gamma.reshape(1, H)
            im["bt"] = beta.reshape(1, H)
        in_maps.append(im)
    meta = (Wc, dest, nti, NI, inv_pad)
    return nc, in_maps, meta


def _run(inputs, trace=False):
    from concourse.bass_utils import run_bass_kernel_spmd

    nc, in_maps, meta = _prep(inputs)
    Wc, dest, nti, NI, inv_pad = meta

    res = run_bass_kernel_spmd(nc, in_maps, core_ids=list(range(NCORES)),
                               trace=trace)

    y_flat = np.empty((B * T, H), np.float32)
    for m in range(NCORES):
        yvm = np.asarray(res.results[m]["yv"]).astype(np.float32)
        dm = dest[m * Wc: (m + 1) * Wc]
        sel = dm >= 0
        y_flat[dm[sel]] = yvm[sel]
        if nti:
            yim = np.asarray(res.results[m]["yi"]).astype(
                np.float32).reshape(NI, H)
            im_idx = inv_pad[m * NI: (m + 1) * NI]
            sel = im_idx >= 0
            y_flat[im_idx[sel]] = yim[sel]

    return y_flat.reshape(B, T, H), res


def kernel(**inputs):
    out, _ = _run(inputs, trace=False)
    return out


# revision 24
# speedup vs baseline: 1.6445x; 1.0033x over previous
"""Trainium2 Bass kernel for AcousticTextEncoderLayer.

Reference computation (B=16, T=4096, H=512, K=9):
  w = weight_norm(weight_v, weight_g)            # per-out-channel scale
  x_masked = hidden_states * (t < len)           # zero beyond each length
  conv = conv1d(x_masked, w, same pad) + bias    # per-sample temporal conv
  y = where(t < len, conv, hidden_states)        # passthrough beyond length
  y = layernorm(y, gamma, beta); leaky_relu(y, 0.1)

Strategy: the per-(b,t) work splits into "valid" positions (t < len: conv +
LN) and "invalid" positions (t >= len: LN only).  The host packs all valid
positions of all samples into one zero-separated timeline (8 zero columns
between samples so the 9-tap conv never mixes samples), splits it evenly
across the 8 cores, and packs invalid rows into equal per-core blocks.
Every core runs the same program on its slice; the host scatters results
back.

Conv runs in fp8e4m3 with DoubleRow matmuls (two 128-deep contraction
tiles per instruction at 0.5 cycles/output-column — 2x the fp16 rate).
Precision is recovered by accumulating, in the same fp32 PSUM group and
at one common scale:
  x_hi.w8  (main, 18 DoubleRow matmuls per 128-position tile)
  x_lo.w8  (exact two-term fp8 split of x: kills the x-side quant error)
  x_hi.s8  (w-residual on MTAPS of the 9 taps: kills most w-side error)
LayerNorm is scale-invariant, so the fp8 scaling (x*16, w*256) and the
matching bias*4096 need no explicit rescale.  Invalid (LN-only) rows ship
as fp16 both ways; valid outputs also return as fp16 (the 2e-2 tolerance
dwarfs fp16 rounding).  LN stats via bn_stats/bn_aggr; normalize + leaky
relu fused into one scalar-engine activation with per-partition
scale/bias (Prelu, alpha=0.1).
"""

import math

import numpy as np
import ml_dtypes

B, T, H, K = 16, 4096, 512, 9
SLOPE = 0.1
EPS = 1e-5
NCORES = 8
SEG = 512          # valid-timeline columns per full segment (4 PSUM tiles)
HALO = K // 2      # 4
SEP = HALO         # zero columns between samples (taps reach <= HALO out)
NPAIR = 2          # DoubleRow chunk pairs (2 x 256 input channels)

SX = 8.0           # fp8 scale for x (LayerNorm absorbs it; small enough
                   # that scaled conv values stay well inside fp16 range)
SW = 256.0         # fp8 scale for w
MTAPS = 4          # taps with w-residual correction (of K)
W_RES_UNITS = 7    # (pair, tap) w-residual matmuls actually emitted
                   # (pr-major order; 7 of the 2*MTAPS=8 possible)
E4 = ml_dtypes.float8_e4m3
WARMUP_MMS = 28    # throwaway matmuls that warm the PE clock during load


def _split_sync_waits(nc, mybir, bass_rust, max_w=1):
    """walrus in this env rejects instructions carrying more than one sync
    wait.  Prefer hoisting extra waits onto the immediately preceding
    same-engine instruction when it has spare wait slots and no sem
    updates (waiting earlier on the same engine is strictly conservative,
    and update-free hosts cannot create wait/update cycles) — this avoids
    the 71ns/inst sequencer cost of a NoOp next to every matmul in the
    weight-arrival window.  Fall back to inserted NoOps otherwise."""
    def n_waits(i):
        return len(i.sync_info.on_wait or []) if i.sync_info is not None else 0

    def can_host(i):
        if i.sync_info is not None and (i.sync_info.on_update or []):
            return False
        return isinstance(i, (mybir.InstLdweights, mybir.InstNoOp))

    for fn in nc.m.functions:
        for bb in fn.blocks:
            out = []
            changed = False
            for inst in bb.instructions:
                si = inst.sync_info
                waits = list(si.on_wait or []) if si is not None else []
                if len(waits) > max_w:
                    extra, keep = waits[:-max_w], waits[-max_w:]
                    # hoist onto the preceding instruction if possible
                    if (extra and out and out[-1].engine == inst.engine
                            and can_host(out[-1])
                            and n_waits(out[-1]) < max_w):
                        prev = out[-1]
                        room = max_w - n_waits(prev)
                        moved, extra = extra[:room], extra[room:]
                        pw = (list(prev.sync_info.on_wait or [])
                              if prev.sync_info is not None else [])
                        prev.sync_info = bass_rust.SyncInfo(
                            on_wait=pw + moved, on_update=[])
                    while extra:
                        chunk, extra = extra[:max_w], extra[max_w:]
                        nop = mybir.InstNoOp(
                            name=nc.get_next_instruction_name(), ins=[], outs=[]
                        )
                        nop.engine = inst.engine
                        nop.sync_info = bass_rust.SyncInfo(
                            on_wait=chunk, on_update=[]
                        )
                        out.append(nop)
                    inst.sync_info = bass_rust.SyncInfo(
                        on_wait=keep, on_update=list(si.on_update or [])
                    )
                    changed = True
                out.append(inst)
            if changed:
                bb.instructions[:] = out


def _build_program(n_sub, nti, apply_gb, repeat=1):
    import concourse.bass as bass
    import concourse.tile as tile
    import concourse.mybir as mybir
    import bass_rust
    from contextlib import ExitStack

    f32 = mybir.dt.float32
    f16 = mybir.dt.float16
    f8 = mybir.dt.float8e4
    DR = mybir.MatmulPerfMode.DoubleRow

    nc = bass.Bass("TRN2", target_bir_lowering=False, debug=False,
                   num_devices=NCORES)
    Wc = n_sub * 128
    # segment widths: full SEG-wide segments plus one remainder
    seg_widths = [SEG] * (Wc // SEG)
    if Wc % SEG:
        seg_widths.append(Wc % SEG)
    nseg = len(seg_widths)
    # x planes: [plane(hi/lo), pair, partition, slot, column]
    xx = nc.dram_tensor("xx", [2, NPAIR, 128, 2, Wc + 2 * HALO], f8,
                        kind="ExternalInput")
    wt = nc.dram_tensor("wt", [NPAIR, 128, K, 2, H], f8, kind="ExternalInput")
    st = None
    if MTAPS:
        st = nc.dram_tensor("st", [NPAIR, 128, MTAPS, 2, H], f8,
                            kind="ExternalInput")
    cb = nc.dram_tensor("cb", [1, H], f16, kind="ExternalInput")
    b8 = nc.dram_tensor("b8", [1, 2, H], f8, kind="ExternalInput")
    yv = nc.dram_tensor("yv", [Wc, H], f16, kind="ExternalOutput")
    xi = yi = None
    if nti:
        xi = nc.dram_tensor("xi", [nti, 128, H], f16, kind="ExternalInput")
        yi = nc.dram_tensor("yi", [nti, 128, H], f16, kind="ExternalOutput")
    gm = bt = None
    if apply_gb:
        gm = nc.dram_tensor("gm", [1, H], f32, kind="ExternalInput")
        bt = nc.dram_tensor("bt", [1, H], f32, kind="ExternalInput")

    AF = mybir.ActivationFunctionType
    OP = mybir.AluOpType

    with tile.TileContext(nc) as tc, ExitStack() as ctx:
        consts = ctx.enter_context(tc.tile_pool(name="consts", bufs=1))
        xpool = ctx.enter_context(tc.tile_pool(name="xpool", bufs=8))
        ipool = ctx.enter_context(tc.tile_pool(name="ipool", bufs=4))
        psum = ctx.enter_context(tc.tile_pool(name="psum", bufs=8, space="PSUM"))
        ypool = ctx.enter_context(tc.tile_pool(name="ypool", bufs=6))
        opool = ctx.enter_context(tc.tile_pool(name="opool", bufs=6))
        spool = ctx.enter_context(tc.tile_pool(name="spool", bufs=8))

        def seg_strips(seg_start, sw, planes=(0, 1), into=None):
            # 4 strips per segment: (plane hi/lo) x (chunk pair), each
            # [128, 2(slot), sw + 2*HALO] fp8.  The slot-dim stride of a
            # DoubleRow ldweights AP must be a multiple of 16 bytes, so the
            # tile width is padded up (the pad columns are never read).
            w_used = sw + 2 * HALO
            w_pad = (w_used + 15) // 16 * 16
            strips = into if into is not None else [None, None]
            for plane in planes:
                row = []
                for pr in range(NPAIR):
                    strip = xpool.tile([128, 2, w_pad], f8,
                                       tag=f"strip{plane}_{pr}")
                    nc.sync.dma_start(
                        out=strip[:, :, 0:w_used],
                        in_=xx[plane, pr, :, :,
                               seg_start: seg_start + w_used])
                    row.append(strip)
                strips[plane] = row
            return strips

        # Startup DMA order matches first-tile consumption order (hi-plane
        # strips, w8 blocks, lo-plane strips, w-residual blocks) — the cost
        # model serializes DMA through one HWDGE device, so order is what
        # hides the load latency behind the PE warmup.  Weights come as one
        # DMA per chunk pair so the PE is not gated on dozens of small
        # transfers.
        strips0 = seg_strips(0, seg_widths[0], planes=(0,))
        wblocks = []
        for pr in range(NPAIR):
            wbl = consts.tile([128, K, 2, H], f8, tag=f"wb{pr}")
            nc.sync.dma_start(out=wbl, in_=wt[pr, :, :, :, :])
            wblocks.append(wbl)
        seg_strips(0, seg_widths[0], planes=(1,), into=strips0)
        # per-pair w-residual tap counts (pr-major split of W_RES_UNITS)
        staps = [min(MTAPS, max(0, W_RES_UNITS - pr * MTAPS))
                 for pr in range(NPAIR)]
        sblocks = []
        for pr in range(NPAIR):
            if not staps[pr]:
                sblocks.append(None)
                continue
            sbl = consts.tile([128, staps[pr], 2, H], f8, tag=f"sb{pr}")
            nc.sync.dma_start(out=sbl, in_=st[pr, :, 0:staps[pr], :, :])
            sblocks.append(sbl)
        wtiles = [[wblocks[pr][:, k, :, :] for k in range(K)]
                  for pr in range(NPAIR)]
        stiles = [[sblocks[pr][:, k, :, :] for k in range(staps[pr])]
                  if sblocks[pr] is not None else []
                  for pr in range(NPAIR)]
        # Warm up the PE clock (HAM gate: 1.2 -> 2.4 GHz after ~3us of
        # sustained activity) with throwaway matmuls on a zeroed tile while
        # the first strips/weights are still in flight.  Results go to a
        # scratch PSUM bank nobody reads.
        if WARMUP_MMS:
            wu_src = consts.tile([128, 2, 128], f8, tag="wu_src")
            nc.vector.memset(wu_src, 0.0)
            wu_mov = consts.tile([128, 2, H], f8, tag="wu_mov")
            nc.vector.memset(wu_mov, 0.0)
            wu_ps = psum.tile([128, H], f32, tag="ps")
            for i in range(WARMUP_MMS):
                nc.tensor.matmul(wu_ps, wu_src, wu_mov,
                                 start=(i == 0), stop=(i == WARMUP_MMS - 1),
                                 perf_mode=DR)

        bias_b = consts.tile([128, H], f16, tag="bias_b")
        nc.sync.dma_start(out=bias_b, in_=cb.ap().to_broadcast((128, H)))
        # bias as a rank-1 fp8 matmul operand (used on the final tile only,
        # to shorten the end-of-program LN chain): one-hot stationary picks
        # partition 0; the two DoubleRow slots carry a hi/lo split of the
        # scaled bias.
        oneh = consts.tile([128, 2, 128], f8, tag="oneh")
        nc.vector.memset(oneh, 0.0)
        nc.vector.memset(oneh[0:1, :, :], 1.0)
        bias8 = consts.tile([128, 2, H], f8, tag="bias8")
        nc.vector.memset(bias8, 0.0)
        nc.sync.dma_start(out=bias8[0:1, :, :], in_=b8.ap())
        gm_b = bt_b = None
        if apply_gb:
            gm_b = consts.tile([128, H], f32, tag="gm_b")
            nc.sync.dma_start(out=gm_b, in_=gm.ap().to_broadcast((128, H)))
            bt_b = consts.tile([128, H], f32, tag="bt_b")
            nc.sync.dma_start(out=bt_b, in_=bt.ap().to_broadcast((128, H)))
        eps_t = consts.tile([128, 1], f32, tag="eps")
        nc.vector.memset(eps_t, EPS)

        def ln_stats(src):
            # per-partition LayerNorm stats: returns (-mean*rstd, rstd)
            stt = spool.tile([128, 6], f32, tag="st")
            nc.vector.bn_stats(out=stt, in_=src)
            mv = spool.tile([128, 2], f32, tag="mv")
            nc.vector.bn_aggr(out=mv, in_=stt)
            sd = spool.tile([128, 1], f32, tag="sd")
            nc.scalar.activation(out=sd, in_=mv[:, 1:2], func=AF.Sqrt,
                                 bias=eps_t, scale=1.0)
            rstd = spool.tile([128, 1], f32, tag="rstd")
            nc.vector.reciprocal(out=rstd, in_=sd)
            nms = spool.tile([128, 1], f32, tag="nms")
            nc.vector.tensor_scalar(out=nms, in0=mv[:, 0:1], scalar1=rstd,
                                    scalar2=-1.0, op0=OP.mult, op1=OP.mult)
            return nms, rstd

        def ln_lrelu(src, dst):
            # LayerNorm over the free dim + leaky relu, into dst.
            nms, rstd = ln_stats(src)
            if not apply_gb:
                nc.scalar.activation(out=dst, in_=src, func=AF.Prelu,
                                     bias=nms, scale=rstd, alpha=SLOPE)
            else:
                tmp = spool.tile([128, H], f32, tag="gbtmp")
                nc.scalar.activation(out=tmp, in_=src, func=AF.Identity,
                                     bias=nms, scale=rstd)
                nc.vector.tensor_mul(out=tmp, in0=tmp, in1=gm_b)
                nc.vector.tensor_add(out=tmp, in0=tmp, in1=bt_b)
                nc.scalar.activation(out=dst, in_=tmp, func=AF.Prelu,
                                     alpha=SLOPE)

        def invalid_tile(it):
            xti = ipool.tile([128, H], f16, tag="xi")
            nc.sync.dma_start(out=xti, in_=xi[it, :, :])
            oi = opool.tile([128, H], f16, tag="oi")
            ln_lrelu(xti, oi)
            nc.sync.dma_start(out=yi[it, :, :], in_=oi)

        # Interleave invalid (LN-only) tiles among valid segments so the
        # vector/scalar engines fill PE-wait gaps.  (repeat>1 re-runs the
        # whole body with identical I/O — used only for differential
        # wall-clock timing, never for the graded kernel.)
        for _rep in range(repeat):
          done_inv = 0
          seg_start = 0
          for s, sw in enumerate(seg_widths):
            if s == 0 and _rep == 0:
                strips = strips0
            else:
                strips = seg_strips(seg_start, sw)
            for sub in range(sw // 128):
                final = (s == nseg - 1 and sub == sw // 128 - 1)
                ps = psum.tile([128, H], f32, tag="ps")
                # one fp32 PSUM accumulation group per tile:
                #   x_hi.w8 (all taps) + x_lo.w8 (all taps) + x_hi.s8
                seq = []
                for pr in range(NPAIR):
                    for k in range(K):
                        seq.append((strips[0][pr], k, wtiles[pr][k]))
                for pr in range(NPAIR):
                    for k in range(K):
                        seq.append((strips[1][pr], k, wtiles[pr][k]))
                for pr in range(NPAIR):
                    for k in range(staps[pr]):
                        seq.append((strips[0][pr], k, stiles[pr][k]))
                if final:
                    # fold the bias add into the PE so the trailing LN chain
                    # (the only one not hidden under matmuls) is shorter
                    seq.append((None, 0, bias8))
                last = len(seq) - 1
                for i, (strip, k, wti) in enumerate(seq):
                    lhsT = (oneh if strip is None else
                            strip[:, :, sub * 128 + k: sub * 128 + k + 128])
                    nc.tensor.matmul(
                        ps, lhsT, wti,
                        start=(i == 0),
                        stop=(i == last),
                        perf_mode=DR,
                    )
                o = opool.tile([128, H], f16, tag="o")
                row0 = seg_start + sub * 128
                if final:
                    ln_lrelu(ps, o)
                else:
                    y = ypool.tile([128, H], f16, tag="y")
                    nc.vector.tensor_add(out=y, in0=ps, in1=bias_b)
                    ln_lrelu(y, o)
                nc.sync.dma_start(out=yv[row0: row0 + 128, :], in_=o)
            seg_start += sw
            inv_target = (0 if s < 2 and s < nseg - 2 else
                          min(nti, s * nti // max(1, nseg - 3)))
            while done_inv < inv_target:
                invalid_tile(done_inv)
                done_inv += 1
        while done_inv < nti:
            invalid_tile(done_inv)
            done_inv += 1

    _split_sync_waits(nc, mybir, bass_rust)
    return nc


def _to_pairs(a):
    """[512, ...cols] -> [pair, 128, slot, ...cols] with
    channel = pair*256 + slot*128 + partition."""
    s = a.shape[1:]
    return np.ascontiguousarray(
        a.reshape(NPAIR, 2, 128, *s).transpose(0, 2, 1, 3))


def _pack(hidden_states, input_lengths):
    """Build per-core packed fp8 inputs + scatter indices."""
    x = np.ascontiguousarray(np.asarray(hidden_states, dtype=np.float32))
    lens = np.asarray(input_lengths).astype(np.int64).clip(0, T)

    starts = np.zeros(B, np.int64)
    col = 0
    for b in range(B):
        starts[b] = col
        col += int(lens[b]) + SEP
    Wt = col
    n_sub = max(1, math.ceil(math.ceil(Wt / NCORES) / 128))
    Wc = n_sub * 128
    Wtot = NCORES * Wc

    XTL = np.zeros((H, HALO + Wtot + HALO), np.float32)
    dest = np.full(Wtot, -1, np.int64)
    for b in range(B):
        L = int(lens[b])
        s0 = int(starts[b])
        XTL[:, HALO + s0: HALO + s0 + L] = x[b, :L, :].T
        dest[s0: s0 + L] = b * T + np.arange(L, dtype=np.int64)

    XTL *= SX
    hi8 = XTL.astype(E4)
    lo8 = (XTL - hi8.astype(np.float32)).astype(E4)

    xts = []
    for m in range(NCORES):
        sl_h = _to_pairs(hi8[:, m * Wc: m * Wc + Wc + 2 * HALO])
        sl_l = _to_pairs(lo8[:, m * Wc: m * Wc + Wc + 2 * HALO])
        xts.append(np.ascontiguousarray(np.stack([sl_h, sl_l])))

    # invalid rows
    inv_mask = (np.arange(T)[None, :] >= lens[:, None]).ravel()
    inv_idx = np.nonzero(inv_mask)[0]
    I = len(inv_idx)
    nti = math.ceil(I / (NCORES * 128)) if I else 0
    NI = nti * 128
    xis = None
    inv_pad = None
    if nti:
        x_flat = x.reshape(B * T, H)
        xi_all = np.zeros((NCORES * NI, H), np.float16)
        xi_all[:I] = x_flat[inv_idx]
        inv_pad = np.full(NCORES * NI, -1, np.int64)
        inv_pad[:I] = inv_idx
        xis = [np.ascontiguousarray(
            xi_all[m * NI: (m + 1) * NI].reshape(nti, 128, H))
            for m in range(NCORES)]

    return x, n_sub, Wc, dest, xts, nti, NI, inv_pad, xis


def _prep_weights(inputs):
    """Quantize weight-norm'ed conv weights to fp8 main + residual."""
    v = np.asarray(inputs["weight_v"], dtype=np.float32)
    g = np.asarray(inputs["weight_g"], dtype=np.float32)
    norm = np.sqrt((v * v).sum(axis=(1, 2), keepdims=True))
    w_eff = (g * v / norm) * SW                           # [H_out, H_in, K]
    wk = np.ascontiguousarray(w_eff.transpose(2, 1, 0))   # [K, H_in, H_out]
    w8 = wk.astype(E4)
    s8 = (wk[:MTAPS] - w8[:MTAPS].astype(np.float32)).astype(E4)
    # device layout: [pair, partition, tap, slot, H_out]
    wt = np.ascontiguousarray(
        np.stack([_to_pairs(w8[k]) for k in range(K)]).transpose(1, 2, 0, 3, 4))
    st = np.ascontiguousarray(
        np.stack([_to_pairs(s8[k]) for k in range(MTAPS)]).transpose(
            1, 2, 0, 3, 4)) if MTAPS else None
    b_s = np.asarray(inputs["conv_bias"], np.float32) * SX * SW
    cb = b_s.astype(np.float16).reshape(1, H)
    # rank-1 fp8 bias for the final-tile fold: hi/lo split keeps the
    # representation error at the fp8-of-residual level
    b_hi = (b_s * 0.5).astype(E4)
    b_lo = (b_s - b_hi.astype(np.float32)).astype(E4)
    b8 = np.ascontiguousarray(
        np.stack([b_hi, b_lo]).reshape(1, 2, H))
    return wt, st, cb, b8


_PROGRAM_CACHE = {}


def _prep(inputs):
    """Pack inputs; return (program, per-core in_maps, scatter metadata)."""
    x, n_sub, Wc, dest, xts, nti, NI, inv_pad, xis = _pack(
        inputs["hidden_states"], inputs["input_lengths"])
    wt, st, cb, b8 = _prep_weights(inputs)
    gamma = np.asarray(inputs["gamma"], np.float32).reshape(H)
    beta = np.asarray(inputs["beta"], np.float32).reshape(H)
    apply_gb = not (np.allclose(gamma, 1.0) and np.allclose(beta, 0.0))

    cache_key = (n_sub, nti, apply_gb)
    nc = _PROGRAM_CACHE.get(cache_key)
    if nc is None:
        nc = _build_program(n_sub, nti, apply_gb)
        _PROGRAM_CACHE[cache_key] = nc

    in_maps = []
    for m in range(NCORES):
        im = {"xx": xts[m], "wt": wt, "cb": cb, "b8": b8}
        if MTAPS:
            im["st"] = st
        if nti:
            im["xi"] = xis[m]
        if apply_gb:
            im["gm"] = gamma.reshape(1, H)
            im["bt"] = beta.reshape(1, H)
        in_maps.append(im)
    meta = (Wc, dest, nti, NI, inv_pad)
    return nc, in_maps, meta


def _run(inputs, trace=False):
    from concourse.bass_utils import run_bass_kernel_spmd

    nc, in_maps, meta = _prep(inputs)
    Wc, dest, nti, NI, inv_pad = meta

    res = run_bass_kernel_spmd(nc, in_maps, core_ids=list(range(NCORES)),
                               trace=trace)

    y_flat = np.empty((B * T, H), np.float32)
    for m in range(NCORES):
        yvm = np.asarray(res.results[m]["yv"]).astype(np.float32)
        dm = dest[m * Wc: (m + 1) * Wc]
        sel = dm >= 0
        y_flat[dm[sel]] = yvm[sel]
        if nti:
            yim = np.asarray(res.results[m]["yi"]).astype(
                np.float32).reshape(NI, H)
            im_idx = inv_pad[m * NI: (m + 1) * NI]
            sel = im_idx >= 0
            y_flat[im_idx[sel]] = yim[sel]

    return y_flat.reshape(B, T, H), res


def kernel(**inputs):
    out, _ = _run(inputs, trace=False)
    return out
